# revision 65
# baseline (speedup 1.0000x reference)
"""GQA causal attention layer (QKV proj + NeoX RoPE + softmax attention + o_proj)
for Trainium2, tensor-parallel over heads across 8 NeuronCores.

Problem shapes (hardcoded): B=1, S=2048, HID=2048, NH=32, NKV=8, HD=64.
Per core c: 4 query heads (4c..4c+3) + 1 kv head (c).

v3 design notes (vs v2 at ~212.6us TimelineSim):
  - Priority-ordered, sliced startup DMAs on the two HWDGE rings (sync/
    scalar); nothing bulk on the gpsimd SWDGE path (SWDGE costs ~1us of
    Pool engine per transfer).  PE warmup dummies cover the initial DMA
    latency so ph1 starts at full clock.
  - Single vaug table [1|v]: PV emits sums at partitions 0:64 (base-0 ->
    reciprocal_approx_fast reads PSUM directly), values at 64:128.
    Normalize = DVE recip + ring-move of rec + DVE mul (PSUM->SBUF bf16),
    chunked at 512 cols so the pv bank frees early.  No Pool muls, no
    SWDGE moves.
  - Diagonal causal masks on Pool (was DVE).
  - m-chunk-1 QKV groups + transposes(1) ride a filler queue inside J0's
    attention (PE never idles while scalar exps run); o_proj mcol=0 jobs
    ride the same queue inside J1 (as in v2).
  - Head order J0 (1,3,0,2) / J1 (0,2,1,3): odd heads first (no kdup
    dep), last J1 head odd (normalize writes outstat directly on the
    critical tail) with 512-col chunked normalize feeding the tail.
"""

import numpy as np

import concourse.bass as bass
import concourse.mybir as mybir
import concourse.tile as tile
from concourse import bacc
from concourse import bass_utils
from concourse.masks import make_identity

B, S, HID = 1, 2048, 2048
NH, NKV, HD = 32, 8, 64
NCORES = 8
HPC = NH // NCORES          # 4 query heads per core
ROPE_BASE = 10000.0
SCALE = 1.0 / np.sqrt(HD)   # 0.125

F32 = mybir.dt.float32
BF16 = mybir.dt.bfloat16
F16 = mybir.dt.float16

KT = S // 128               # 16 k-tiles of 128
MC = 1024                   # phase-1 m-chunk
QCHUNK = 1024               # attention q-chunk
J0_ORDER = (1, 3, 0, 2)
J1_ORDER = (0, 2, 1, 3)
N_WARMUP = 48

PHASE_BOUNDS = []           # [(name, n_matmuls)] filled at build time


def build_kernel(passes=1, debug_dump=False):
    nc = bacc.Bacc("TRN2", target_bir_lowering=False, debug=False,
                   num_devices=NCORES)

    xA = nc.dram_tensor("xA", [128, KT, MC], BF16, kind="ExternalInput").ap()
    xB = nc.dram_tensor("xB", [128, KT, MC], BF16, kind="ExternalInput").ap()
    w3 = nc.dram_tensor("w3", [128, KT, 384], BF16, kind="ExternalInput").ap()
    wo = nc.dram_tensor("wo", [256, HID], BF16, kind="ExternalInput").ap()
    Cr = nc.dram_tensor("Cr", [128, S], BF16, kind="ExternalInput").ap()
    Sr = nc.dram_tensor("Sr", [128, S], BF16, kind="ExternalInput").ap()
    trimask = nc.dram_tensor("trimask", [128, 128], BF16,
                             kind="ExternalInput").ap()
    yA = nc.dram_tensor("yA", [128, KT, MC], F16, kind="ExternalOutput").ap()
    yB = nc.dram_tensor("yB", [128, KT, MC], F16, kind="ExternalOutput").ap()
    dbg = {}
    if debug_dump:
        for nm, shp, dt in [("dqr0", [128, S], BF16), ("dqr1", [128, S], BF16),
                            ("dkr", [128, S], BF16), ("dva", [128, S], BF16),
                            ("dos0", [128, S], BF16), ("dos1", [128, S], BF16)]:
            dbg[nm] = nc.dram_tensor(nm, shp, dt, kind="ExternalOutput").ap()

    PHASE_BOUNDS.clear()
    mm_count = [0]

    def phase(name):
        PHASE_BOUNDS.append([name, mm_count[0]])
        mm_count[0] = 0

    def MM(*args, **kwargs):
        mm_count[0] += 1
        return nc.tensor.matmul(*args, **kwargs)

    def TP(*args, **kwargs):
        mm_count[0] += 1
        return nc.tensor.transpose(*args, **kwargs)

    with tile.TileContext(nc) as tc:
      for _pass in range(passes):
        with (
            tc.tile_pool(name="persist", bufs=1, side=None) as pers,
            tc.tile_pool(name="xpool", bufs=1) as xpool,
        ):
            # ---- persistent tiles ----
            qr = [pers.tile([128, S], BF16, tag=f"qr{t}", name=f"qr{t}")
                  for t in range(2)]
            kr = pers.tile([128, S], BF16, tag="kr")
            outstat = [pers.tile([128, S], BF16, tag=f"os{p}", name=f"os{p}")
                       for p in range(2)]
            wo_sb = [pers.tile([128, HID], BF16, tag=f"wo{p}", name=f"wo{p}")
                     for p in range(2)]
            Ct = pers.tile([128, S], BF16, tag="Ct")
            St = pers.tile([128, S], BF16, tag="St")
            wsb = pers.tile([128, KT * 384], BF16, tag="wsb")
            vaug = pers.tile([128, KT, 128], BF16, tag="vaug")
            trim = pers.tile([128, 128], BF16, tag="trim")
            ident = pers.tile([128, 128], BF16, tag="ident")
            wup = pers.tile([128, 128], BF16, tag="wup")

            xb0 = [xpool.tile([128, 4 * MC], BF16, tag=f"xb0_{b}",
                              name=f"xb0_{b}") for b in range(4)]
            xb1 = [xpool.tile([128, 4 * MC], BF16, tag=f"xb1_{b}",
                              name=f"xb1_{b}") for b in range(4)]

            # ---- preload DMAs: priority order IS service order (the DMA
            # engine pool serializes); slice the head of the stream so the
            # first matmuls can start ~4us in.  All bulk on HWDGE rings. ----
            W4 = 4 * 384
            nc.sync.dma_start(wsb[:, 0:W4], w3[:, 0:4, :])
            nc.scalar.dma_start(xb0[0][:, 0:2 * MC], xA[:, 0:2, :])
            nc.sync.dma_start(wsb[:, W4:2 * W4], w3[:, 4:8, :])
            nc.scalar.dma_start(xb0[0][:, 2 * MC:4 * MC], xA[:, 2:4, :])
            nc.sync.dma_start(xb0[1], xA[:, 4:8, :])
            nc.scalar.dma_start(wsb[:, 2 * W4:4 * W4], w3[:, 8:16, :])
            nc.sync.dma_start(xb0[2], xA[:, 8:12, :])
            nc.sync.dma_start(xb1[0], xB[:, 0:4, :])
            nc.scalar.dma_start(xb0[3], xA[:, 12:16, :])
            nc.scalar.dma_start(xb1[1], xB[:, 4:8, :])
            nc.sync.dma_start(xb1[2][:, 0:2 * MC], xB[:, 8:10, :])
            nc.scalar.dma_start(xb1[2][:, 2 * MC:4 * MC], xB[:, 10:12, :])
            nc.sync.dma_start(xb1[3][:, 0:2 * MC], xB[:, 12:14, :])
            nc.scalar.dma_start(xb1[3][:, 2 * MC:4 * MC], xB[:, 14:16, :])
            nc.sync.dma_start(Ct, Cr)
            nc.scalar.dma_start(St, Sr)
            nc.sync.dma_start(trim, trimask)
            for p in range(2):
                nc.scalar.dma_start(wo_sb[p], wo[128 * p:128 * (p + 1), :])
            make_identity(nc, ident)
            nc.gpsimd.memset(vaug[:, :, 0:64], 1.0)

            # ====== pools ======
            qkv6 = tc.alloc_tile_pool(name="qkv6", bufs=1, space="PSUM")
            ps3 = [qkv6.tile([128, MC], F32, tag=f"ps3_{n}", name=f"ps3_{n}")
                   for n in range(3)]
            ps_n2 = [qkv6.tile([128, 512], F32, tag=f"m1n2_{c}",
                               name=f"m1n2_{c}") for c in range(2)]

            # ---- PE warmup: keep the clock hot while startup DMAs land
            # (vector memset is ready almost immediately) ----
            nc.vector.memset(wup, 0.0)
            for d in range(N_WARMUP):
                MM(ps3[0][:, 0:128], wup, wup, start=True, stop=True)
            phase("warmup")

            # ====== phase 1: QKV projection, m-chunk 0 ======
            NORD_LAST = (2, 0, 1)
            for b in range(8):
                for k in (2 * b, 2 * b + 1):
                    nord = NORD_LAST if k == KT - 1 else (0, 1, 2)
                    for n in nord:
                        for c in range(2):
                            MM(ps3[n][:, 512 * c:512 * (c + 1)],
                               wsb[:, 384 * k + 128 * n:
                                   384 * k + 128 * (n + 1)],
                               xb0[b // 2][:, (k % 4) * MC + 512 * c:
                                           (k % 4) * MC + 512 * (c + 1)],
                               start=(k == 0), stop=(k == KT - 1))
            phase("ph1_qkv")

            ev0 = tc.alloc_tile_pool(name="ev0", bufs=1)
            ev1 = tc.alloc_tile_pool(name="ev1", bufs=1)
            swp = tc.alloc_tile_pool(name="swp", bufs=3)
            qkvb0 = [ev0.tile([128, MC], BF16, tag=f"qkvb0_{t}",
                              name=f"qkvb0_{t}") for t in range(3)]
            qkvb1 = [ev1.tile([128, MC], BF16, tag=f"qkvb1_{t}",
                              name=f"qkvb1_{t}") for t in range(3)]

            def rope_muls(t, m0, qkvb):
                """dst = qkv*C ; qbS = qkv*S ; swap-dma qbS -> sw (ring)."""
                r0, r1 = (0, 128) if t < 2 else (64, 128)
                dst = qr[t] if t < 2 else kr
                qbS = swp.tile([128, MC], BF16, tag=f"qbS",
                               name=f"qbS{t}_{m0}")
                sw = swp.tile([128, MC], BF16, tag=f"sw",
                              name=f"sw{t}_{m0}")
                nc.vector.tensor_mul(dst[r0:r1, m0:m0 + MC],
                                     qkvb[t][r0:r1, 0:MC],
                                     Ct[r0:r1, m0:m0 + MC])
                nc.gpsimd.tensor_mul(qbS[r0:r1, 0:MC], qkvb[t][r0:r1, 0:MC],
                                     St[r0:r1, m0:m0 + MC])
                for g in range(r0 // 32, r1 // 32, 2):
                    nc.sync.dma_start(sw[32 * g:32 * g + 32, :],
                                      qbS[32 * g + 32:32 * g + 64, :])
                    nc.sync.dma_start(sw[32 * g + 32:32 * g + 64, :],
                                      qbS[32 * g:32 * g + 32, :])
                return sw

            def rope_add(t, m0, sw, kdup_eng=None):
                r0, r1 = (0, 128) if t < 2 else (64, 128)
                dst = qr[t] if t < 2 else kr
                nc.vector.tensor_add(dst[r0:r1, m0:m0 + MC],
                                     dst[r0:r1, m0:m0 + MC], sw[r0:r1, :])
                if t == 2:
                    (kdup_eng or nc.sync).dma_start(kr[0:64, m0:m0 + MC],
                                                    kr[64:128, m0:m0 + MC])

            def qkv_m1_unit(n, c, kq, ps, evict=True):
                """4 accumulating matmuls (quarter k-tiles) for m-chunk 1."""
                for k in range(4 * kq, 4 * kq + 4):
                    MM(ps[:, 0:512],
                       wsb[:, 384 * k + 128 * n:384 * k + 128 * (n + 1)],
                       xb1[k // 4][:, (k % 4) * MC + 512 * c:
                                   (k % 4) * MC + 512 * (c + 1)],
                       start=(k == 0), stop=(k == KT - 1))
                if evict and kq == 3:
                    nc.vector.tensor_copy(qkvb1[n][:, 512 * c:512 * (c + 1)],
                                          ps[:, 0:512])

            # ---- m-chunk-0 evict + rope (vector/scalar/pool/rings), while
            # the PE runs the m-chunk-1 k/v group inline (qkv6 psum) ----
            nc.vector.tensor_copy(qkvb0[2], ps3[2])       # k|v first
            sw_k = rope_muls(2, 0, qkvb0)
            nc.scalar.copy(qkvb0[0], ps3[0])
            sw_q0 = rope_muls(0, 0, qkvb0)
            nc.scalar.copy(qkvb0[1], ps3[1])
            sw_q1 = rope_muls(1, 0, qkvb0)
            for kq in range(4):                            # inline m1 k|v
                for c in range(2):
                    qkv_m1_unit(2, c, kq, ps_n2[c], evict=False)
            for c in range(2):
                nc.scalar.copy(qkvb1[2][:, 512 * c:512 * (c + 1)],
                               ps_n2[c][:, 0:512])
            phase("m1_n2")

            qkv6.release()

            # ====== attention pools (after qkv6 release: 8 PSUM banks).
            # fill sits at the bottom of the stack so stp+pvp can be
            # released before the tail for a deeper psum rotation. ======
            fill = tc.alloc_tile_pool(name="fill", bufs=2, space="PSUM")
            stp = tc.alloc_tile_pool(name="stp", bufs=2, space="PSUM")
            pvp = tc.alloc_tile_pool(name="pvp", bufs=1, space="PSUM")
            ptp = tc.alloc_tile_pool(name="ptp", bufs=4)
            nrm = tc.alloc_tile_pool(name="nrm", bufs=2)
            ysbp = tc.alloc_tile_pool(name="ysbp", bufs=8)

            def transposes(mc, qkvb2):
                """v^T into vaug[:, i, 64:128]: 8 transposes into one PSUM
                tile back-to-back, then a single strided DVE copy."""
                tpb = fill.tile([128, 8, 64], BF16, tag="fl", name=f"tpb{mc}")
                for i in range(8):
                    TP(tpb[:, i, :], qkvb2[0:64, 128 * i:128 * (i + 1)],
                       ident[0:64, 0:64])
                nc.vector.tensor_copy(vaug[:, 8 * mc:8 * (mc + 1), 64:128],
                                      tpb)

            rope_add(2, 0, sw_k)
            rope_add(0, 0, sw_q0)
            rope_add(1, 0, sw_q1)
            transposes(0, qkvb0[2])
            phase("transposes0")

            # ---- filler queue consumed inside J0's attention; ordered so
            # everything J1's first heads need comes out early ----
            fillq = [
                lambda: rope_add(2, MC, rope_muls(2, MC, qkvb1),
                                 kdup_eng=nc.scalar),
                lambda: transposes(1, qkvb1[2]),
            ]

            def m1_units(n):
                for c in range(2):
                    ps_h = {}

                    def unit(kq, n=n, c=c, ps_h=ps_h):
                        if kq == 0:
                            ps_h[0] = fill.tile([128, 512], F32, tag="fl",
                                                name=f"m1_{n}_{c}")
                        qkv_m1_unit(n, c, kq, ps_h[0])

                    for kq in range(4):
                        fillq.append(lambda u=unit, kq=kq: u(kq))

            m1_units(0)
            fillq.append(lambda: rope_add(0, MC, rope_muls(0, MC, qkvb1)))
            m1_units(1)
            fillq.append(lambda: rope_add(1, MC, rope_muls(1, MC, qkvb1)))

            def filler(_slot):
                if fillq:
                    fillq.pop(0)()

            def norm_chunk(j, h, pv, cc, tiles, move_eng=None,
                           fast_tail=False):
                """Normalize cols [512cc:512cc+512] of pv.  Emitted inline
                as soon as the last PV block writing those cols is issued
                (cols 0:512c+512 are final after i = 8j + 4cc + 3)."""
                half, p = h % 2, h // 2
                me = move_eng or nc.gpsimd
                q0 = QCHUNK * j
                rec, pvs, tmp = tiles
                sl = slice(512 * cc, 512 * (cc + 1))
                osl = slice(q0 + 512 * cc, q0 + 512 * (cc + 1))
                nc.vector.reciprocal_approx_fast(rec[0:64, sl], pv[0:64, sl])
                me.dma_start(rec[64:128, sl], rec[0:64, sl])
                if fast_tail:
                    assert half
                    nc.vector.tensor_mul(outstat[p][64:128, osl],
                                         pv[64:128, sl], rec[64:128, sl])
                    return
                nc.vector.tensor_copy(pvs[64:128, sl], pv[64:128, sl])
                if half:
                    nc.gpsimd.tensor_mul(outstat[p][64:128, osl],
                                         pvs[64:128, sl], rec[64:128, sl])
                else:
                    nc.gpsimd.tensor_mul(tmp[64:128, sl],
                                         pvs[64:128, sl], rec[64:128, sl])
                    me.dma_start(outstat[p][0:64, osl], tmp[64:128, sl])

            def emit_attn(j, h, absorber=None, norm_move=None,
                          fast_tail=False):
                half, p = h % 2, h // 2
                qrow = 64 * half
                kb = 64 * half
                ilast = 8 * (j + 1) - 1
                pv = pvp.tile([128, QCHUNK], F32, tag="pv", name=f"pv{j}_{h}")
                rec = nrm.tile([128, QCHUNK], F32, tag="rec",
                               name=f"rec{j}_{h}")
                pvs = tmp = None
                pvs = nrm.tile([128, QCHUNK], BF16, tag="pvs",
                               name=f"pvs{j}_{h}")
                if not half:
                    tmp = nrm.tile([128, QCHUNK], BF16, tag="tmp",
                                   name=f"tmp{j}_{h}")
                ntile = (rec, pvs, tmp)
                # triangle blocks packed in complementary pairs so one exp
                # call covers a full [128,1024] tile (fewer scalar-engine
                # fixed overheads; scalar paces J1)
                if j == 0:
                    groups = [[0], [1, 7], [2, 6], [3, 5], [4]]
                    slots = {1, 2, 3, 4}
                    norm0_g = 3          # group after which cols 0:512 final
                else:
                    groups = [[i] for i in range(9)] + \
                        [[9, 15], [10, 14], [11, 13], [12]]
                    slots = {1, 3, 5, 7, 9, 10, 11, 12}
                    norm0_g = 11
                ab = 0

                def seg_chunks(a, b):
                    """split [a,b) at multiples of 512 (psum bank bounds)"""
                    out = []
                    while a < b:
                        e = min(b, (a // 512 + 1) * 512)
                        out.append((a, e - a))
                        a = e
                    return out

                def emit_pvs(gi, metas, pt):
                    for (i, o0, qstart, qlen) in metas:
                        diag = 128 * i >= QCHUNK * j
                        off = qstart - QCHUNK * j
                        chunks = seg_chunks(off, off + qlen)
                        # emit the non-masked chunk first (hides mask latency)
                        if diag and len(chunks) > 1:
                            chunks = chunks[::-1]
                        for (a, cols) in chunks:
                            MM(pv[:, a:a + cols],
                               vaug[:, i, :],
                               pt[:, o0 + (a - off):o0 + (a - off) + cols],
                               start=(gi == 0), stop=(gi == len(groups) - 1))
                    if gi == norm0_g:
                        norm_chunk(j, h, pv, 0, ntile, move_eng=norm_move)
                    elif gi == len(groups) - 1:
                        norm_chunk(j, h, pv, 1, ntile, move_eng=norm_move,
                                   fast_tail=fast_tail)

                pending = None   # PV emission runs one group behind QK/exp
                for gi, grp in enumerate(groups):
                    metas = []
                    ot = 0
                    st = stp.tile([128, 1024], F32, tag="st",
                                  name=f"st{j}_{h}_{gi}")
                    pt = ptp.tile([128, 1024], BF16, tag="pt",
                                  name=f"pt{j}_{h}_{gi}")
                    for i in grp:
                        qstart = max(QCHUNK * j, 128 * i)
                        qlen = QCHUNK * (j + 1) - qstart
                        for (a, cols) in seg_chunks(ot, ot + qlen):
                            MM(st[:, a:a + cols],
                               kr[kb:kb + 64, 128 * i:128 * (i + 1)],
                               qr[p][qrow:qrow + 64,
                                     qstart + (a - ot):
                                     qstart + (a - ot) + cols],
                               start=True, stop=True)
                        metas.append((i, ot, qstart, qlen))
                        ot += qlen
                    nc.scalar.activation(
                        pt[:, 0:ot], st[:, 0:ot],
                        mybir.ActivationFunctionType.Exp, scale=SCALE)
                    mask_eng = nc.gpsimd if j == 1 else nc.vector
                    for (i, o0, qstart, qlen) in metas:
                        if 128 * i >= QCHUNK * j:
                            mask_eng.tensor_mul(pt[:, o0:o0 + 128],
                                                pt[:, o0:o0 + 128], trim)
                    if pending is not None:
                        emit_pvs(*pending)
                    pending = (gi, metas, pt)
                    if absorber is not None and gi in slots:
                        absorber(ab)
                        ab += 1
                emit_pvs(*pending)

            # ====== J0: attention q-chunk 0, fillers interleaved ======
            for hi, h in enumerate(J0_ORDER):
                emit_attn(0, h, absorber=filler,
                          norm_move=(nc.sync if hi == 3 else None))
                phase(f"J0_h{h}")
            while fillq:
                fillq.pop(0)()
            phase("J0_fill_rest")

            # ====== J1 with o_proj mcol=0 interleaved ======
            ysb_jobs = {}

            def oproj_job(nt, c, mcol, cp_eng, store_eng, pool=None,
                          half_store=False):
                """One [128,512] o_proj chunk: 2 matmuls + fp16 copy + store."""
                pso = (pool or fill).tile([128, 512], F32,
                                          tag=("fl" if (pool or fill) is fill
                                               else "tl"),
                                          name=f"pso{nt}_{c}_{mcol}")
                for p in range(2):
                    MM(pso[:, 0:512],
                       wo_sb[p][:, 128 * nt:128 * (nt + 1)],
                       outstat[p][:, mcol + 512 * c:mcol + 512 * (c + 1)],
                       start=(p == 0), stop=(p == 1))
                pair, slot = nt // 2, nt % 2
                ysbt = ysb_jobs.get((pair, mcol))
                if ysbt is None:
                    ysbt = ysbp.tile([128, 2, MC], F16, tag="ysb",
                                     name=f"ysb{pair}_{mcol}")
                    ysb_jobs[(pair, mcol)] = ysbt
                if cp_eng is nc.scalar:
                    nc.scalar.copy(ysbt[:, slot, 512 * c:512 * (c + 1)],
                                   pso[:, 0:512])
                else:
                    cp_eng.tensor_copy(ysbt[:, slot, 512 * c:512 * (c + 1)],
                                       pso[:, 0:512])
                ydst = yA if mcol == 0 else yB
                if half_store:
                    # c-major sweeps: store this 512-col half of the pair as
                    # soon as both slots have it.
                    if slot == 1:
                        store_eng.dma_start(
                            ydst[:, 2 * pair:2 * pair + 2,
                                 512 * c:512 * (c + 1)],
                            ysbt[:, :, 512 * c:512 * (c + 1)])
                        if c == 1:
                            del ysb_jobs[(pair, mcol)]
                elif slot == 1 and c == 1:
                    store_eng.dma_start(ydst[:, 2 * pair:2 * pair + 2, :],
                                        ysbt)
                    del ysb_jobs[(pair, mcol)]

            for hi, h in enumerate(J1_ORDER):
                jobs = [(nt, c)
                        for nt in range(4 * hi, 4 * hi + 4) for c in range(2)]

                def absorber(ab, jobs=jobs, hi=hi):
                    # hi==0: outstat j0 cols land late (J0's last heads);
                    # don't let an early oproj mm block the PE FIFO.
                    # hi==3: evict on scalar so DVE is free for the last
                    # head's low-latency normalize; keep the last 4 jobs to
                    # run during the normalize latency (PE stays warm).
                    ev = nc.vector
                    if hi == 0:
                        if ab < 4:
                            return
                        for jb in (2 * (ab - 4), 2 * (ab - 4) + 1):
                            nt, c = jobs[jb]
                            oproj_job(nt, c, 0, ev, nc.sync)
                    elif hi == 3 and ab >= 4:
                        return
                    else:
                        nt, c = jobs[ab]
                        oproj_job(nt, c, 0, ev, nc.sync)

                emit_attn(1, h, absorber=absorber,
                          norm_move=(nc.sync if hi == 3 else None),
                          fast_tail=(hi == 3))
                if hi == 3:
                    for jb in range(4, 8):
                        nt, c = jobs[jb]
                        oproj_job(nt, c, 0, nc.scalar, nc.sync)
                phase(f"J1_h{h}")

            # ====== o_proj mcol=1024 tail: release pvp (its boundary waits
            # only the last pv readers) -> 2 extra banks; 4-deep rotation
            # across fill+tailp.  c-major sweeps (c=0 jobs only need the
            # last head's first normalize chunk); half-stores per sweep so
            # the output DMA is spread, not bunched at the end.  The first
            # two jobs ride fill (no release boundary) so the PE never
            # stalls at the sweep start. ======
            pvp.release()
            tailp = tc.alloc_tile_pool(name="tailp", bufs=2, space="PSUM")

            for c in range(2):
                for nt in range(KT):
                    pool = fill if (nt < 2 or nt % 2 == 1) else tailp
                    oproj_job(nt, c, MC,
                              nc.scalar if nt % 2 == 0 else nc.vector,
                              nc.sync, pool=pool, half_store=True)
            phase("tail_oproj")

            if debug_dump:
                nc.sync.dma_start(dbg["dqr0"], qr[0])
                nc.sync.dma_start(dbg["dqr1"], qr[1])
                nc.sync.dma_start(dbg["dkr"], kr)
                nc.sync.dma_start(dbg["dva"], vaug)
                nc.sync.dma_start(dbg["dos0"], outstat[0])
                nc.sync.dma_start(dbg["dos1"], outstat[1])

            ysbp.release()
            nrm.release()
            ptp.release()
            tailp.release()
            stp.release()
            fill.release()
            swp.release()
            ev1.release()
            ev0.release()

    nc.compile()
    return nc


def make_host_inputs(x, w_qkv, w_o):
    """Host-side prep: tiled/transposed bf16 inputs, rope tables."""
    import ml_dtypes
    bf16 = ml_dtypes.bfloat16
    x = np.asarray(x, dtype=np.float32)
    w_qkv = np.asarray(w_qkv, dtype=np.float32)
    w_o = np.asarray(w_o, dtype=np.float32)
    xT = np.ascontiguousarray(x.reshape(S, HID).T)          # [HID, S]
    xT3 = xT.reshape(KT, 128, S).transpose(1, 0, 2)
    xAh = np.ascontiguousarray(xT3[:, :, 0:MC]).astype(bf16)
    xBh = np.ascontiguousarray(xT3[:, :, MC:S]).astype(bf16)

    inv_freq = 1.0 / (ROPE_BASE ** (np.arange(0, HD, 2, dtype=np.float32) / HD))
    t = np.arange(S, dtype=np.float32)
    freqs = np.outer(t, inv_freq)                     # [S, 32]
    cosT = np.cos(freqs).T.astype(np.float32)         # [32, S]
    sinT = np.sin(freqs).T.astype(np.float32)
    C = np.ascontiguousarray(np.tile(cosT, (4, 1))).astype(bf16)   # [128, S]
    # S is applied BEFORE the row swap, so the sign pattern rides along with
    # the swap: rows 0:32 = +sin, 32:64 = -sin (swapped vs the classic table).
    Sn = np.ascontiguousarray(np.tile(np.concatenate([sinT, -sinT], 0),
                                      (2, 1))).astype(bf16)

    r = np.arange(128)
    trimask = np.where(r[None, :] >= r[:, None], np.float32(1.0),
                       np.float32(0.0)).astype(bf16)

    in_maps = []
    for c in range(NCORES):
        qcols = np.arange(4 * c * HD, 4 * (c + 1) * HD)
        vcols = NH * HD + NKV * HD + np.arange(c * HD, (c + 1) * HD)
        kcols = NH * HD + np.arange(c * HD, (c + 1) * HD)
        w_stat = np.concatenate(
            [w_qkv[:, qcols], w_qkv[:, vcols], w_qkv[:, kcols]], axis=1)
        w3c = np.ascontiguousarray(
            w_stat.reshape(KT, 128, 384).transpose(1, 0, 2)).astype(bf16)
        wo_c = np.ascontiguousarray(
            w_o[256 * c:256 * (c + 1), :]).astype(bf16)
        in_maps.append({
            "xA": xAh, "xB": xBh, "w3": w3c, "wo": wo_c,
            "Cr": C, "Sr": Sn, "trimask": trimask,
        })
    return in_maps


_NC_CACHE = {}


def get_nc():
    if "nc" not in _NC_CACHE:
        _NC_CACHE["nc"] = build_kernel()
    return _NC_CACHE["nc"]


def _get_exec():
    """Build (once) the jitted sharded executable over the 8 cores."""
    if "exec" in _NC_CACHE:
        return _NC_CACHE["exec"]
    import jax
    from jax.sharding import Mesh, PartitionSpec, NamedSharding
    from jax.experimental.shard_map import shard_map
    from concourse import bass2jax

    nc = get_nc()
    bass2jax.install_neuronx_cc_hook()
    partition_name = (nc.partition_id_tensor.name
                      if nc.partition_id_tensor else None)
    in_names, out_names, out_avals, zero_outs = [], [], [], []
    for alloc in nc.m.functions[0].allocations:
        if not isinstance(alloc, mybir.MemoryLocationSet):
            continue
        name = alloc.memorylocations[0].name
        if alloc.kind == "ExternalInput":
            if name != partition_name:
                in_names.append(name)
        elif alloc.kind == "ExternalOutput":
            shape = tuple(alloc.tensor_shape)
            dtype = mybir.dt.np(alloc.dtype)
            out_names.append(name)
            out_avals.append(jax.core.ShapedArray(shape, dtype))
            zero_outs.append(np.zeros(shape, dtype))
    n_params = len(in_names)
    all_in = list(in_names) + list(out_names)
    if partition_name is not None:
        all_in.append(partition_name)

    def _body(*args):
        operands = list(args)
        if partition_name is not None:
            operands.append(bass2jax.partition_id_tensor())
        return tuple(bass2jax._bass_exec_p.bind(
            *operands, out_avals=tuple(out_avals), in_names=tuple(all_in),
            out_names=tuple(out_names), lowering_input_output_aliases=(),
            sim_require_finite=True, sim_require_nnan=True, nc=nc))

    devices = jax.devices()[:NCORES]
    mesh = Mesh(np.asarray(devices), ("core",))
    REPL = {"xA", "xB", "Cr", "Sr", "trimask"}
    in_specs = tuple(PartitionSpec() if n in REPL else PartitionSpec("core")
                     for n in in_names)
    in_specs = in_specs + (PartitionSpec("core"),) * len(out_names)
    f = jax.jit(shard_map(_body, mesh=mesh, in_specs=in_specs,
                          out_specs=(PartitionSpec("core"),) * len(out_names),
                          check_rep=False), keep_unused=True)
    sh = NamedSharding(mesh, PartitionSpec("core"))
    shr = NamedSharding(mesh, PartitionSpec())
    _NC_CACHE["exec"] = (f, in_names, out_names, zero_outs, sh, shr, REPL)
    return _NC_CACHE["exec"]


def kernel(x, w_qkv, w_o):
    import jax

    f, in_names, out_names, zero_outs, sh, shr, REPL = _get_exec()
    in_maps = make_host_inputs(x, w_qkv, w_o)
    args = []
    for name in in_names:
        if name in REPL:
            args.append(jax.device_put(in_maps[0][name], shr))
        else:
            args.append(jax.device_put(
                np.concatenate([m[name] for m in in_maps], 0), sh))
    if "zeros" not in _NC_CACHE:
        _NC_CACHE["zeros"] = [
            jax.device_put(
                np.zeros((NCORES * z.shape[0], *z.shape[1:]), z.dtype), sh)
            for z in zero_outs]
    args += _NC_CACHE["zeros"]
    outs = f(*args)
    ya_idx = out_names.index("yA")
    yb_idx = out_names.index("yB")
    if "reduce" not in _NC_CACHE:
        import jax.numpy as jnp

        def _reduce(a, b):
            # a,b: [8*128, KT, MC] fp16 partials -> [S, HID] f32
            y = jnp.concatenate(
                [jnp.reshape(a, (NCORES, 128, KT, MC)),
                 jnp.reshape(b, (NCORES, 128, KT, MC))], axis=3)
            y = jnp.sum(y.astype(jnp.float32), axis=0)     # [128, KT, S]
            y = jnp.transpose(y, (1, 0, 2))                # [KT, 128, S]
            return jnp.transpose(jnp.reshape(y, (HID, S)))  # [S, HID]
        _NC_CACHE["reduce"] = jax.jit(_reduce)
    out = np.asarray(_NC_CACHE["reduce"](outs[ya_idx], outs[yb_idx]))
    return np.ascontiguousarray(out.astype(np.float32)).reshape(B, S, HID)


# revision 82
# speedup vs baseline: 1.0371x; 1.0371x over previous
"""GQA causal attention layer (QKV proj + NeoX RoPE + softmax attention + o_proj)
for Trainium2, tensor-parallel over heads across 8 NeuronCores.

Problem shapes (hardcoded): B=1, S=2048, HID=2048, NH=32, NKV=8, HD=64.
Per core c: 4 query heads (4c..4c+3) + 1 kv head (c).

v3 design notes (vs v2 at ~212.6us TimelineSim):
  - Priority-ordered, sliced startup DMAs on the two HWDGE rings (sync/
    scalar); nothing bulk on the gpsimd SWDGE path (SWDGE costs ~1us of
    Pool engine per transfer).  PE warmup dummies cover the initial DMA
    latency so ph1 starts at full clock.
  - Single vaug table [1|v]: PV emits sums at partitions 0:64 (base-0 ->
    reciprocal_approx_fast reads PSUM directly), values at 64:128.
    Normalize = DVE recip + ring-move of rec + DVE mul (PSUM->SBUF bf16),
    chunked at 512 cols so the pv bank frees early.  No Pool muls, no
    SWDGE moves.
  - Diagonal causal masks on Pool (was DVE).
  - m-chunk-1 QKV groups + transposes(1) ride a filler queue inside J0's
    attention (PE never idles while scalar exps run); o_proj mcol=0 jobs
    ride the same queue inside J1 (as in v2).
  - Head order J0 (1,3,0,2) / J1 (0,2,1,3): odd heads first (no kdup
    dep), last J1 head odd (normalize writes outstat directly on the
    critical tail) with 512-col chunked normalize feeding the tail.
"""

import numpy as np

import concourse.bass as bass
import concourse.mybir as mybir
import concourse.tile as tile
from concourse import bacc
from concourse import bass_utils
from concourse.masks import make_identity

B, S, HID = 1, 2048, 2048
NH, NKV, HD = 32, 8, 64
NCORES = 8
HPC = NH // NCORES          # 4 query heads per core
ROPE_BASE = 10000.0
SCALE = 1.0 / np.sqrt(HD)   # 0.125

F32 = mybir.dt.float32
BF16 = mybir.dt.bfloat16
F16 = mybir.dt.float16

KT = S // 128               # 16 k-tiles of 128
MC = 1024                   # phase-1 m-chunk
QCHUNK = 1024               # attention q-chunk
J0_ORDER = (1, 3, 0, 2)
J1_ORDER = (0, 2, 1, 3)
N_WARMUP = 48

PHASE_BOUNDS = []           # [(name, n_matmuls)] filled at build time


def build_kernel(passes=1, debug_dump=False):
    nc = bacc.Bacc("TRN2", target_bir_lowering=False, debug=False,
                   num_devices=NCORES)

    xA = nc.dram_tensor("xA", [128, KT, MC], BF16, kind="ExternalInput").ap()
    xB = nc.dram_tensor("xB", [128, KT, MC], BF16, kind="ExternalInput").ap()
    w3 = nc.dram_tensor("w3", [128, KT, 384], BF16, kind="ExternalInput").ap()
    wo = nc.dram_tensor("wo", [256, HID], BF16, kind="ExternalInput").ap()
    Cr = nc.dram_tensor("Cr", [128, S], BF16, kind="ExternalInput").ap()
    Sr = nc.dram_tensor("Sr", [128, S], BF16, kind="ExternalInput").ap()
    trimask = nc.dram_tensor("trimask", [128, 128], BF16,
                             kind="ExternalInput").ap()
    swapmr = nc.dram_tensor("swapm", [128, 128], BF16,
                            kind="ExternalInput").ap()
    yA = nc.dram_tensor("yA", [128, KT, MC], F16, kind="ExternalOutput").ap()
    yB = nc.dram_tensor("yB", [128, KT, MC], F16, kind="ExternalOutput").ap()
    dbg = {}
    if debug_dump:
        for nm, shp, dt in [("dqr0", [128, S], BF16), ("dqr1", [128, S], BF16),
                            ("dkr", [128, S], BF16), ("dva", [128, S], BF16),
                            ("dos0", [128, S], BF16), ("dos1", [128, S], BF16)]:
            dbg[nm] = nc.dram_tensor(nm, shp, dt, kind="ExternalOutput").ap()

    PHASE_BOUNDS.clear()
    mm_count = [0]

    def phase(name):
        PHASE_BOUNDS.append([name, mm_count[0]])
        mm_count[0] = 0

    def MM(*args, **kwargs):
        mm_count[0] += 1
        return nc.tensor.matmul(*args, **kwargs)

    def TP(*args, **kwargs):
        mm_count[0] += 1
        return nc.tensor.transpose(*args, **kwargs)

    with tile.TileContext(nc) as tc:
      for _pass in range(passes):
        with (
            tc.tile_pool(name="persist", bufs=1, side=None) as pers,
            tc.tile_pool(name="xpool", bufs=1) as xpool,
        ):
            # ---- persistent tiles ----
            qr = [pers.tile([128, S], BF16, tag=f"qr{t}", name=f"qr{t}")
                  for t in range(2)]
            kr = pers.tile([128, S], BF16, tag="kr")
            outstat = [pers.tile([128, S], BF16, tag=f"os{p}", name=f"os{p}")
                       for p in range(2)]
            wo_sb = [pers.tile([128, HID], BF16, tag=f"wo{p}", name=f"wo{p}")
                     for p in range(2)]
            Ct = pers.tile([128, S], BF16, tag="Ct")
            St = pers.tile([128, S], BF16, tag="St")
            wsb = pers.tile([128, KT * 384], BF16, tag="wsb")
            vaug = pers.tile([128, KT, 128], BF16, tag="vaug")
            trim = pers.tile([128, 128], BF16, tag="trim")
            swapm = pers.tile([128, 128], BF16, tag="swapm")
            ident = pers.tile([128, 128], BF16, tag="ident")
            wup = pers.tile([128, 128], BF16, tag="wup")

            xb0 = [xpool.tile([128, 4 * MC], BF16, tag=f"xb0_{b}",
                              name=f"xb0_{b}") for b in range(4)]
            xb1 = [xpool.tile([128, 4 * MC], BF16, tag=f"xb1_{b}",
                              name=f"xb1_{b}") for b in range(4)]

            # ---- preload DMAs: priority order IS service order (the DMA
            # engine pool serializes); slice the head of the stream so the
            # first matmuls can start ~4us in.  All bulk on HWDGE rings. ----
            W4 = 4 * 384
            nc.sync.dma_start(wsb[:, 0:W4], w3[:, 0:4, :])
            nc.scalar.dma_start(xb0[0][:, 0:2 * MC], xA[:, 0:2, :])
            nc.sync.dma_start(wsb[:, W4:2 * W4], w3[:, 4:8, :])
            nc.scalar.dma_start(xb0[0][:, 2 * MC:4 * MC], xA[:, 2:4, :])
            nc.sync.dma_start(xb0[1], xA[:, 4:8, :])
            nc.scalar.dma_start(wsb[:, 2 * W4:4 * W4], w3[:, 8:16, :])
            nc.sync.dma_start(xb0[2], xA[:, 8:12, :])
            nc.sync.dma_start(xb1[0], xB[:, 0:4, :])
            nc.scalar.dma_start(xb0[3], xA[:, 12:16, :])
            nc.scalar.dma_start(xb1[1], xB[:, 4:8, :])
            nc.sync.dma_start(Ct[:, 0:MC], Cr[:, 0:MC])
            nc.scalar.dma_start(St[:, 0:MC], Sr[:, 0:MC])
            nc.scalar.dma_start(swapm, swapmr)
            nc.sync.dma_start(xb1[2][:, 0:2 * MC], xB[:, 8:10, :])
            nc.scalar.dma_start(xb1[2][:, 2 * MC:4 * MC], xB[:, 10:12, :])
            nc.sync.dma_start(xb1[3][:, 0:2 * MC], xB[:, 12:14, :])
            # everything below rides the scalar ring so the sync queue is
            # empty when the rope-swap DMAs arrive (device round-robins
            # between queue heads)
            nc.scalar.dma_start(xb1[3][:, 2 * MC:4 * MC], xB[:, 14:16, :])
            nc.scalar.dma_start(trim, trimask)
            nc.scalar.dma_start(Ct[:, MC:S], Cr[:, MC:S])
            nc.scalar.dma_start(St[:, MC:S], Sr[:, MC:S])
            for p in range(2):
                nc.scalar.dma_start(wo_sb[p], wo[128 * p:128 * (p + 1), :])
            make_identity(nc, ident)
            nc.gpsimd.memset(vaug[:, :, 0:64], 1.0)

            # ====== pools ======
            qkv6 = tc.alloc_tile_pool(name="qkv6", bufs=1, space="PSUM")
            ps3 = [qkv6.tile([128, MC], F32, tag=f"ps3_{n}", name=f"ps3_{n}")
                   for n in range(3)]
            ps_n2 = [qkv6.tile([128, 512], F32, tag=f"m1n2_{c}",
                               name=f"m1n2_{c}") for c in range(2)]

            # ---- PE warmup: keep the clock hot while startup DMAs land
            # (vector memset is ready almost immediately) ----
            nc.vector.memset(wup, 0.0)
            for d in range(N_WARMUP):
                MM(ps3[0][:, 0:128], wup, wup, start=True, stop=True)
            phase("warmup")

            # ====== phase 1: QKV projection, m-chunk 0 ======
            NORD_LAST = (2, 0, 1)
            for b in range(8):
                for k in (2 * b, 2 * b + 1):
                    nord = NORD_LAST if k == KT - 1 else (0, 1, 2)
                    for n in nord:
                        for c in range(2):
                            MM(ps3[n][:, 512 * c:512 * (c + 1)],
                               wsb[:, 384 * k + 128 * n:
                                   384 * k + 128 * (n + 1)],
                               xb0[b // 2][:, (k % 4) * MC + 512 * c:
                                           (k % 4) * MC + 512 * (c + 1)],
                               start=(k == 0), stop=(k == KT - 1))
            phase("ph1_qkv")

            ev0 = tc.alloc_tile_pool(name="ev0", bufs=1)
            ev1 = tc.alloc_tile_pool(name="ev1", bufs=1)
            swp = tc.alloc_tile_pool(name="swp", bufs=3)
            qkvb0 = [ev0.tile([128, MC], BF16, tag=f"qkvb0_{t}",
                              name=f"qkvb0_{t}") for t in range(3)]
            qkvb1 = [ev1.tile([128, MC], BF16, tag=f"qkvb1_{t}",
                              name=f"qkvb1_{t}") for t in range(3)]

            def rope_muls(t, m0, qkvb, pe=False):
                """dst = qkv*C ; qbS = qkv*S ; swap via ring dma (default)
                or deferred to the PE (pe=True -> rope_add_pe)."""
                r0, r1 = (0, 128) if t < 2 else (64, 128)
                dst = qr[t] if t < 2 else kr
                qbS = swp.tile([128, MC], BF16, tag=f"qbS",
                               name=f"qbS{t}_{m0}")
                nc.vector.tensor_mul(dst[r0:r1, m0:m0 + MC],
                                     qkvb[t][r0:r1, 0:MC],
                                     Ct[r0:r1, m0:m0 + MC])
                # full 128 rows even for k (rows 0:64 = v * sin, discarded):
                # keeps the PE-swap moving operand fully initialized
                nc.gpsimd.tensor_mul(qbS[:, 0:MC], qkvb[t][:, 0:MC],
                                     St[:, m0:m0 + MC])
                if pe:
                    return qbS
                sw = swp.tile([128, MC], BF16, tag=f"sw",
                              name=f"sw{t}_{m0}")
                for g in range(r0 // 32, r1 // 32, 2):
                    nc.sync.dma_start(sw[32 * g:32 * g + 32, :],
                                      qbS[32 * g + 32:32 * g + 64, :])
                    nc.sync.dma_start(sw[32 * g + 32:32 * g + 64, :],
                                      qbS[32 * g:32 * g + 32, :])
                return sw

            def rope_add(t, m0, sw, kdup_eng=None):
                r0, r1 = (0, 128) if t < 2 else (64, 128)
                dst = qr[t] if t < 2 else kr
                nc.vector.tensor_add(dst[r0:r1, m0:m0 + MC],
                                     dst[r0:r1, m0:m0 + MC], sw[r0:r1, :])
                if t == 2:
                    (kdup_eng or nc.sync).dma_start(kr[0:64, m0:m0 + MC],
                                                    kr[64:128, m0:m0 + MC])

            def rope_add_pe(t, m0, qbS, targets, kdup_eng=None):
                """swap on the PE (permutation matmul into freed ph1 psum) +
                DVE add from PSUM — keeps the startup rope off the congested
                DMA device."""
                r0, r1 = (0, 128) if t < 2 else (64, 128)
                dst = qr[t] if t < 2 else kr
                for c in range(2):
                    sw = targets[c]
                    MM(sw, swapm, qbS[:, 512 * c:512 * (c + 1)],
                       start=True, stop=True)
                    nc.vector.tensor_add(
                        dst[r0:r1, m0 + 512 * c:m0 + 512 * (c + 1)],
                        dst[r0:r1, m0 + 512 * c:m0 + 512 * (c + 1)],
                        sw[r0:r1, 0:512])
                if t == 2:
                    (kdup_eng or nc.sync).dma_start(kr[0:64, m0:m0 + MC],
                                                    kr[64:128, m0:m0 + MC])

            def qkv_m1_unit(n, c, kq, ps, evict=True):
                """4 accumulating matmuls (quarter k-tiles) for m-chunk 1."""
                for k in range(4 * kq, 4 * kq + 4):
                    MM(ps[:, 0:512],
                       wsb[:, 384 * k + 128 * n:384 * k + 128 * (n + 1)],
                       xb1[k // 4][:, (k % 4) * MC + 512 * c:
                                   (k % 4) * MC + 512 * (c + 1)],
                       start=(k == 0), stop=(k == KT - 1))
                if evict and kq == 3:
                    nc.vector.tensor_copy(qkvb1[n][:, 512 * c:512 * (c + 1)],
                                          ps[:, 0:512])

            # ---- m-chunk-0 evict + rope (vector/scalar/pool), interleaved
            # with the m-chunk-1 k/v group on the PE.  The rope swaps are
            # permutation matmuls into ph1 psum banks freed by the evicts —
            # nothing rides the (saturated) DMA device. ----
            nc.vector.tensor_copy(qkvb0[2], ps3[2])       # k|v first
            sw_k = rope_muls(2, 0, qkvb0, pe=True)
            nc.scalar.copy(qkvb0[0], ps3[0])
            sw_q0 = rope_muls(0, 0, qkvb0, pe=True)
            nc.scalar.copy(qkvb0[1], ps3[1])
            sw_q1 = rope_muls(1, 0, qkvb0, pe=True)
            for c in range(2):
                qkv_m1_unit(2, c, 0, ps_n2[c], evict=False)
            rope_add_pe(2, 0, sw_k,
                        [ps3[2][:, 0:512], ps3[2][:, 512:1024]])
            for c in range(2):
                qkv_m1_unit(2, c, 1, ps_n2[c], evict=False)
            rope_add_pe(0, 0, sw_q0,
                        [ps3[0][:, 0:512], ps3[0][:, 512:1024]])
            for c in range(2):
                qkv_m1_unit(2, c, 2, ps_n2[c], evict=False)
            rope_add_pe(1, 0, sw_q1,
                        [ps3[1][:, 0:512], ps3[1][:, 512:1024]])
            for c in range(2):
                qkv_m1_unit(2, c, 3, ps_n2[c], evict=False)
            for c in range(2):
                nc.vector.tensor_copy(qkvb1[2][:, 512 * c:512 * (c + 1)],
                                      ps_n2[c][:, 0:512])
            phase("m1_n2")

            qkv6.release()

            # ====== attention pools (after qkv6 release: 8 PSUM banks).
            # fill sits at the bottom of the stack so stp+pvp can be
            # released before the tail for a deeper psum rotation. ======
            fill = tc.alloc_tile_pool(name="fill", bufs=2, space="PSUM")
            stp = tc.alloc_tile_pool(name="stp", bufs=2, space="PSUM")
            pvp = tc.alloc_tile_pool(name="pvp", bufs=1, space="PSUM")
            ptp = tc.alloc_tile_pool(name="ptp", bufs=4)
            nrm = tc.alloc_tile_pool(name="nrm", bufs=2)
            ysbp = tc.alloc_tile_pool(name="ysbp", bufs=8)

            def transposes(mc, qkvb2):
                """v^T into vaug[:, i, 64:128]: 8 transposes into one PSUM
                tile back-to-back, then a single strided DVE copy."""
                tpb = fill.tile([128, 8, 64], BF16, tag="fl", name=f"tpb{mc}")
                for i in range(8):
                    TP(tpb[:, i, :], qkvb2[0:64, 128 * i:128 * (i + 1)],
                       ident[0:64, 0:64])
                nc.vector.tensor_copy(vaug[:, 8 * mc:8 * (mc + 1), 64:128],
                                      tpb)

            # ---- filler queue consumed inside J0's attention; ordered so
            # everything J1's first heads need comes out early ----
            fillq = [
                lambda: transposes(0, qkvb0[2]),
                lambda: rope_add(2, MC, rope_muls(2, MC, qkvb1),
                                 kdup_eng=nc.scalar),
                lambda: transposes(1, qkvb1[2]),
            ]

            def m1_units(n):
                for c in range(2):
                    ps_h = {}

                    def unit(kq, n=n, c=c, ps_h=ps_h):
                        if kq == 0:
                            ps_h[0] = fill.tile([128, 512], F32, tag="fl",
                                                name=f"m1_{n}_{c}")
                        qkv_m1_unit(n, c, kq, ps_h[0])

                    for kq in range(4):
                        fillq.append(lambda u=unit, kq=kq: u(kq))

            m1_units(0)
            fillq.append(lambda: rope_add(0, MC, rope_muls(0, MC, qkvb1)))
            m1_units(1)
            fillq.append(lambda: rope_add(1, MC, rope_muls(1, MC, qkvb1)))

            def filler(_slot):
                if fillq:
                    fillq.pop(0)()

            def norm_chunk(j, h, pv, cc, tiles, move_eng=None,
                           fast_tail=False):
                """Normalize cols [512cc:512cc+512] of pv.  Emitted inline
                as soon as the last PV block writing those cols is issued
                (cols 0:512c+512 are final after i = 8j + 4cc + 3)."""
                half, p = h % 2, h // 2
                me = move_eng or nc.gpsimd
                q0 = QCHUNK * j
                rec, pvs, tmp = tiles
                sl = slice(512 * cc, 512 * (cc + 1))
                osl = slice(q0 + 512 * cc, q0 + 512 * (cc + 1))
                nc.vector.reciprocal_approx_fast(rec[0:64, sl], pv[0:64, sl])
                me.dma_start(rec[64:128, sl], rec[0:64, sl])
                if fast_tail:
                    assert half
                    nc.vector.tensor_mul(outstat[p][64:128, osl],
                                         pv[64:128, sl], rec[64:128, sl])
                    return
                nc.vector.tensor_copy(pvs[64:128, sl], pv[64:128, sl])
                if half:
                    nc.gpsimd.tensor_mul(outstat[p][64:128, osl],
                                         pvs[64:128, sl], rec[64:128, sl])
                else:
                    nc.gpsimd.tensor_mul(tmp[64:128, sl],
                                         pvs[64:128, sl], rec[64:128, sl])
                    me.dma_start(outstat[p][0:64, osl], tmp[64:128, sl])

            def emit_attn(j, h, absorber=None, norm_move=None,
                          fast_tail=False):
                half, p = h % 2, h // 2
                qrow = 64 * half
                kb = 64 * half
                ilast = 8 * (j + 1) - 1
                pv = pvp.tile([128, QCHUNK], F32, tag="pv", name=f"pv{j}_{h}")
                rec = nrm.tile([128, QCHUNK], F32, tag="rec",
                               name=f"rec{j}_{h}")
                pvs = tmp = None
                pvs = nrm.tile([128, QCHUNK], BF16, tag="pvs",
                               name=f"pvs{j}_{h}")
                if not half:
                    tmp = nrm.tile([128, QCHUNK], BF16, tag="tmp",
                                   name=f"tmp{j}_{h}")
                ntile = (rec, pvs, tmp)
                # triangle blocks packed in complementary pairs so one exp
                # call covers a full [128,1024] tile (fewer scalar-engine
                # fixed overheads; scalar paces J1)
                if j == 0:
                    groups = [[0], [1, 7], [2, 6], [3, 5], [4]]
                    slots = {1, 2, 3, 4}
                    norm0_g = 3          # group after which cols 0:512 final
                else:
                    groups = [[i] for i in range(9)] + \
                        [[9, 15], [10, 14], [11, 13], [12]]
                    slots = {1, 3, 5, 7, 9, 10, 11, 12}
                    norm0_g = 11
                ab = 0

                def seg_chunks(a, b):
                    """split [a,b) at multiples of 512 (psum bank bounds)"""
                    out = []
                    while a < b:
                        e = min(b, (a // 512 + 1) * 512)
                        out.append((a, e - a))
                        a = e
                    return out

                def emit_pvs(gi, metas, pt):
                    for (i, o0, qstart, qlen) in metas:
                        diag = 128 * i >= QCHUNK * j
                        off = qstart - QCHUNK * j
                        chunks = seg_chunks(off, off + qlen)
                        # emit the non-masked chunk first (hides mask latency)
                        if diag and len(chunks) > 1:
                            chunks = chunks[::-1]
                        for (a, cols) in chunks:
                            MM(pv[:, a:a + cols],
                               vaug[:, i, :],
                               pt[:, o0 + (a - off):o0 + (a - off) + cols],
                               start=(gi == 0), stop=(gi == len(groups) - 1))
                    if gi == norm0_g:
                        norm_chunk(j, h, pv, 0, ntile, move_eng=norm_move)
                    elif gi == len(groups) - 1:
                        norm_chunk(j, h, pv, 1, ntile, move_eng=norm_move,
                                   fast_tail=fast_tail)

                pending = None   # PV emission runs one group behind QK/exp
                for gi, grp in enumerate(groups):
                    metas = []
                    ot = 0
                    st = stp.tile([128, 1024], F32, tag="st",
                                  name=f"st{j}_{h}_{gi}")
                    pt = ptp.tile([128, 1024], BF16, tag="pt",
                                  name=f"pt{j}_{h}_{gi}")
                    for i in grp:
                        qstart = max(QCHUNK * j, 128 * i)
                        qlen = QCHUNK * (j + 1) - qstart
                        for (a, cols) in seg_chunks(ot, ot + qlen):
                            MM(st[:, a:a + cols],
                               kr[kb:kb + 64, 128 * i:128 * (i + 1)],
                               qr[p][qrow:qrow + 64,
                                     qstart + (a - ot):
                                     qstart + (a - ot) + cols],
                               start=True, stop=True)
                        metas.append((i, ot, qstart, qlen))
                        ot += qlen
                    nc.scalar.activation(
                        pt[:, 0:ot], st[:, 0:ot],
                        mybir.ActivationFunctionType.Exp, scale=SCALE)
                    for (i, o0, qstart, qlen) in metas:
                        if 128 * i >= QCHUNK * j:
                            nc.vector.tensor_mul(pt[:, o0:o0 + 128],
                                                 pt[:, o0:o0 + 128], trim)
                    if absorber is not None and gi in slots:
                        absorber(ab)
                        ab += 1
                    if pending is not None:
                        emit_pvs(*pending)
                    pending = (gi, metas, pt)
                emit_pvs(*pending)

            # ====== J0: attention q-chunk 0, fillers interleaved ======
            for hi, h in enumerate(J0_ORDER):
                emit_attn(0, h, absorber=filler,
                          norm_move=(nc.sync if hi == 3 else None))
                phase(f"J0_h{h}")
            while fillq:
                fillq.pop(0)()
            phase("J0_fill_rest")

            # ====== J1 with o_proj mcol=0 interleaved ======
            ysb_jobs = {}

            def oproj_job(nt, c, mcol, cp_eng, store_eng, pool=None,
                          half_store=False):
                """One [128,512] o_proj chunk: 2 matmuls + fp16 copy + store."""
                pso = (pool or fill).tile([128, 512], F32,
                                          tag=("fl" if (pool or fill) is fill
                                               else "tl"),
                                          name=f"pso{nt}_{c}_{mcol}")
                for p in range(2):
                    MM(pso[:, 0:512],
                       wo_sb[p][:, 128 * nt:128 * (nt + 1)],
                       outstat[p][:, mcol + 512 * c:mcol + 512 * (c + 1)],
                       start=(p == 0), stop=(p == 1))
                pair, slot = nt // 2, nt % 2
                ysbt = ysb_jobs.get((pair, mcol))
                if ysbt is None:
                    ysbt = ysbp.tile([128, 2, MC], F16, tag="ysb",
                                     name=f"ysb{pair}_{mcol}")
                    ysb_jobs[(pair, mcol)] = ysbt
                if cp_eng is nc.scalar:
                    nc.scalar.copy(ysbt[:, slot, 512 * c:512 * (c + 1)],
                                   pso[:, 0:512])
                else:
                    cp_eng.tensor_copy(ysbt[:, slot, 512 * c:512 * (c + 1)],
                                       pso[:, 0:512])
                ydst = yA if mcol == 0 else yB
                if half_store:
                    # c-major sweeps: store this 512-col half of the pair as
                    # soon as both slots have it.
                    if slot == 1:
                        store_eng.dma_start(
                            ydst[:, 2 * pair:2 * pair + 2,
                                 512 * c:512 * (c + 1)],
                            ysbt[:, :, 512 * c:512 * (c + 1)])
                        if c == 1:
                            del ysb_jobs[(pair, mcol)]
                elif slot == 1 and c == 1:
                    store_eng.dma_start(ydst[:, 2 * pair:2 * pair + 2, :],
                                        ysbt)
                    del ysb_jobs[(pair, mcol)]

            for hi, h in enumerate(J1_ORDER):
                jobs = [(nt, c)
                        for nt in range(4 * hi, 4 * hi + 4) for c in range(2)]

                def absorber(ab, jobs=jobs, hi=hi):
                    # hi==0: outstat j0 cols land late (J0's last heads);
                    # don't let an early oproj mm block the PE FIFO.
                    # hi==3: evict on scalar so DVE is free for the last
                    # head's low-latency normalize; keep the last 4 jobs to
                    # run during the normalize latency (PE stays warm).
                    ev = nc.vector
                    if hi == 0:
                        if ab < 4:
                            return
                        for jb in (2 * (ab - 4), 2 * (ab - 4) + 1):
                            nt, c = jobs[jb]
                            oproj_job(nt, c, 0, ev, nc.sync)
                    elif hi == 3 and ab >= 4:
                        return
                    else:
                        nt, c = jobs[ab]
                        oproj_job(nt, c, 0, ev, nc.sync)

                emit_attn(1, h, absorber=absorber,
                          norm_move=(nc.sync if hi == 3 else None),
                          fast_tail=(hi == 3))
                if hi == 3:
                    for jb in range(4, 8):
                        nt, c = jobs[jb]
                        oproj_job(nt, c, 0, nc.scalar, nc.sync)
                phase(f"J1_h{h}")

            # ====== o_proj mcol=1024 tail: release pvp (its boundary waits
            # only the last pv readers) -> 2 extra banks; 4-deep rotation
            # across fill+tailp.  c-major sweeps (c=0 jobs only need the
            # last head's first normalize chunk); half-stores per sweep so
            # the output DMA is spread, not bunched at the end.  The first
            # two jobs ride fill (no release boundary) so the PE never
            # stalls at the sweep start. ======
            pvp.release()
            tailp = tc.alloc_tile_pool(name="tailp", bufs=2, space="PSUM")

            for c in range(2):
                for nt in range(KT):
                    pool = fill if (nt < 2 or nt % 2 == 1) else tailp
                    oproj_job(nt, c, MC,
                              nc.scalar if nt % 2 == 0 else nc.vector,
                              nc.sync, pool=pool, half_store=True)
            phase("tail_oproj")

            if debug_dump:
                nc.sync.dma_start(dbg["dqr0"], qr[0])
                nc.sync.dma_start(dbg["dqr1"], qr[1])
                nc.sync.dma_start(dbg["dkr"], kr)
                nc.sync.dma_start(dbg["dva"], vaug)
                nc.sync.dma_start(dbg["dos0"], outstat[0])
                nc.sync.dma_start(dbg["dos1"], outstat[1])

            ysbp.release()
            nrm.release()
            ptp.release()
            tailp.release()
            stp.release()
            fill.release()
            swp.release()
            ev1.release()
            ev0.release()

    nc.compile()
    return nc


def make_host_inputs(x, w_qkv, w_o):
    """Host-side prep: tiled/transposed bf16 inputs, rope tables."""
    import ml_dtypes
    bf16 = ml_dtypes.bfloat16
    x = np.asarray(x, dtype=np.float32)
    w_qkv = np.asarray(w_qkv, dtype=np.float32)
    w_o = np.asarray(w_o, dtype=np.float32)
    xT = np.ascontiguousarray(x.reshape(S, HID).T)          # [HID, S]
    xT3 = xT.reshape(KT, 128, S).transpose(1, 0, 2)
    xAh = np.ascontiguousarray(xT3[:, :, 0:MC]).astype(bf16)
    xBh = np.ascontiguousarray(xT3[:, :, MC:S]).astype(bf16)

    inv_freq = 1.0 / (ROPE_BASE ** (np.arange(0, HD, 2, dtype=np.float32) / HD))
    t = np.arange(S, dtype=np.float32)
    freqs = np.outer(t, inv_freq)                     # [S, 32]
    cosT = np.cos(freqs).T.astype(np.float32)         # [32, S]
    sinT = np.sin(freqs).T.astype(np.float32)
    C = np.ascontiguousarray(np.tile(cosT, (4, 1))).astype(bf16)   # [128, S]
    # S is applied BEFORE the row swap, so the sign pattern rides along with
    # the swap: rows 0:32 = +sin, 32:64 = -sin (swapped vs the classic table).
    Sn = np.ascontiguousarray(np.tile(np.concatenate([sinT, -sinT], 0),
                                      (2, 1))).astype(bf16)

    r = np.arange(128)
    trimask = np.where(r[None, :] >= r[:, None], np.float32(1.0),
                       np.float32(0.0)).astype(bf16)
    swapm = np.where(r[None, :] == (r[:, None] ^ 32), np.float32(1.0),
                     np.float32(0.0)).astype(bf16)

    in_maps = []
    for c in range(NCORES):
        qcols = np.arange(4 * c * HD, 4 * (c + 1) * HD)
        vcols = NH * HD + NKV * HD + np.arange(c * HD, (c + 1) * HD)
        kcols = NH * HD + np.arange(c * HD, (c + 1) * HD)
        w_stat = np.concatenate(
            [w_qkv[:, qcols], w_qkv[:, vcols], w_qkv[:, kcols]], axis=1)
        w3c = np.ascontiguousarray(
            w_stat.reshape(KT, 128, 384).transpose(1, 0, 2)).astype(bf16)
        wo_c = np.ascontiguousarray(
            w_o[256 * c:256 * (c + 1), :]).astype(bf16)
        in_maps.append({
            "xA": xAh, "xB": xBh, "w3": w3c, "wo": wo_c,
            "Cr": C, "Sr": Sn, "trimask": trimask, "swapm": swapm,
        })
    return in_maps


_NC_CACHE = {}


def get_nc():
    if "nc" not in _NC_CACHE:
        _NC_CACHE["nc"] = build_kernel()
    return _NC_CACHE["nc"]


def _get_exec():
    """Build (once) the jitted sharded executable over the 8 cores."""
    if "exec" in _NC_CACHE:
        return _NC_CACHE["exec"]
    import jax
    from jax.sharding import Mesh, PartitionSpec, NamedSharding
    from jax.experimental.shard_map import shard_map
    from concourse import bass2jax

    nc = get_nc()
    bass2jax.install_neuronx_cc_hook()
    partition_name = (nc.partition_id_tensor.name
                      if nc.partition_id_tensor else None)
    in_names, out_names, out_avals, zero_outs = [], [], [], []
    for alloc in nc.m.functions[0].allocations:
        if not isinstance(alloc, mybir.MemoryLocationSet):
            continue
        name = alloc.memorylocations[0].name
        if alloc.kind == "ExternalInput":
            if name != partition_name:
                in_names.append(name)
        elif alloc.kind == "ExternalOutput":
            shape = tuple(alloc.tensor_shape)
            dtype = mybir.dt.np(alloc.dtype)
            out_names.append(name)
            out_avals.append(jax.core.ShapedArray(shape, dtype))
            zero_outs.append(np.zeros(shape, dtype))
    n_params = len(in_names)
    all_in = list(in_names) + list(out_names)
    if partition_name is not None:
        all_in.append(partition_name)

    def _body(*args):
        operands = list(args)
        if partition_name is not None:
            operands.append(bass2jax.partition_id_tensor())
        return tuple(bass2jax._bass_exec_p.bind(
            *operands, out_avals=tuple(out_avals), in_names=tuple(all_in),
            out_names=tuple(out_names), lowering_input_output_aliases=(),
            sim_require_finite=True, sim_require_nnan=True, nc=nc))

    devices = jax.devices()[:NCORES]
    mesh = Mesh(np.asarray(devices), ("core",))
    REPL = {"xA", "xB", "Cr", "Sr", "trimask", "swapm"}
    in_specs = tuple(PartitionSpec() if n in REPL else PartitionSpec("core")
                     for n in in_names)
    in_specs = in_specs + (PartitionSpec("core"),) * len(out_names)
    f = jax.jit(shard_map(_body, mesh=mesh, in_specs=in_specs,
                          out_specs=(PartitionSpec("core"),) * len(out_names),
                          check_rep=False), keep_unused=True)
    sh = NamedSharding(mesh, PartitionSpec("core"))
    shr = NamedSharding(mesh, PartitionSpec())
    _NC_CACHE["exec"] = (f, in_names, out_names, zero_outs, sh, shr, REPL)
    return _NC_CACHE["exec"]


def kernel(x, w_qkv, w_o):
    import jax

    f, in_names, out_names, zero_outs, sh, shr, REPL = _get_exec()
    in_maps = make_host_inputs(x, w_qkv, w_o)
    args = []
    for name in in_names:
        if name in REPL:
            args.append(jax.device_put(in_maps[0][name], shr))
        else:
            args.append(jax.device_put(
                np.concatenate([m[name] for m in in_maps], 0), sh))
    if "zeros" not in _NC_CACHE:
        _NC_CACHE["zeros"] = [
            jax.device_put(
                np.zeros((NCORES * z.shape[0], *z.shape[1:]), z.dtype), sh)
            for z in zero_outs]
    args += _NC_CACHE["zeros"]
    outs = f(*args)
    ya_idx = out_names.index("yA")
    yb_idx = out_names.index("yB")
    if "reduce" not in _NC_CACHE:
        import jax.numpy as jnp

        def _reduce(a, b):
            # a,b: [8*128, KT, MC] fp16 partials -> [S, HID] f32
            y = jnp.concatenate(
                [jnp.reshape(a, (NCORES, 128, KT, MC)),
                 jnp.reshape(b, (NCORES, 128, KT, MC))], axis=3)
            y = jnp.sum(y.astype(jnp.float32), axis=0)     # [128, KT, S]
            y = jnp.transpose(y, (1, 0, 2))                # [KT, 128, S]
            return jnp.transpose(jnp.reshape(y, (HID, S)))  # [S, HID]
        _NC_CACHE["reduce"] = jax.jit(_reduce)
    out = np.asarray(_NC_CACHE["reduce"](outs[ya_idx], outs[yb_idx]))
    return np.ascontiguousarray(out.astype(np.float32)).reshape(B, S, HID)


# revision 104
# speedup vs baseline: 1.0800x; 1.0414x over previous
"""GQA causal attention layer (QKV proj + NeoX RoPE + softmax attention + o_proj)
for Trainium2, tensor-parallel over heads across 8 NeuronCores.

Problem shapes (hardcoded): B=1, S=2048, HID=2048, NH=32, NKV=8, HD=64.
Per core c: 4 query heads (4c..4c+3) + 1 kv head (c).

v3 design notes (~157us TimelineSim vs v2's ~212.6us; HW rel err 6.0e-3):
  - Priority-ordered, sliced startup DMAs on the two HWDGE rings (the
    sim's DMA device serializes in issue order; ~2us fixed latency per
    transfer).  Nothing bulk on gpsimd SWDGE (costs ~1us Pool engine per
    transfer).  PE warmup dummies on a memset tile keep the clock ramp
    hot until ph1's first data lands.
  - m-chunk-0 rope is DMA-free: qbS = qkv*S on Pool, the 32-row half
    swap is a permutation matmul (swapm) into ph1 psum banks freed by
    the evictions, add on DVE from PSUM.  Interleaved with the m1 k|v
    group on the PE.  m-chunk-1 rope keeps ring-dma swaps (device is
    idle by then).
  - Single vaug table [1|v] as a 3D [128,KT,128] tile: PV emits sums at
    partitions 0:64 (base-0 -> reciprocal_approx_fast reads PSUM
    directly), values at 64:128.  v^T lands via 8 batched PE transposes
    + one strided DVE copy per m-chunk.
  - Normalize is emitted inline per 512-col chunk as soon as the last
    PV block writing those columns is issued (chunk c is final after
    group 8j+4c+3) — the o_proj tail starts with no normalize latency.
    Last J1 head (odd) writes outstat directly from PSUM on DVE.
  - Triangle blocks packed in complementary pairs per exp call (J1: 13
    calls/head vs 16; scalar activation has ~370ns fixed cost and paces
    J1).  PV emission runs one group behind QK so a waiting PV never
    blocks the next QK in the PE FIFO; absorber fillers fire before the
    pending PV flush.
  - Filler queue inside J0: transposes, m1 rope chains, m1 QKV quarter
    groups; o_proj mcol=0 jobs ride J1's slots (delayed for the first
    head; partially post-attention for the last).  c-major o_proj tail
    with half-stores, fed by the released pvp bank + fill rotation.
  - Head order J0 (1,3,0,2) / J1 (0,2,1,3): odd heads first (no kdup
    dep), last J1 head odd.
"""

import numpy as np

import concourse.bass as bass
import concourse.mybir as mybir
import concourse.tile as tile
from concourse import bacc
from concourse import bass_utils
from concourse.masks import make_identity

B, S, HID = 1, 2048, 2048
NH, NKV, HD = 32, 8, 64
NCORES = 8
HPC = NH // NCORES          # 4 query heads per core
ROPE_BASE = 10000.0
SCALE = 1.0 / np.sqrt(HD)   # 0.125

F32 = mybir.dt.float32
BF16 = mybir.dt.bfloat16
F16 = mybir.dt.float16

KT = S // 128               # 16 k-tiles of 128
MC = 1024                   # phase-1 m-chunk
QCHUNK = 1024               # attention q-chunk
J0_ORDER = (1, 3, 0, 2)
J1_ORDER = (0, 2, 1, 3)
N_WARMUP = 48

PHASE_BOUNDS = []           # [(name, n_matmuls)] filled at build time


def build_kernel(passes=1, debug_dump=False):
    nc = bacc.Bacc("TRN2", target_bir_lowering=False, debug=False,
                   num_devices=NCORES)

    xA = nc.dram_tensor("xA", [128, KT, MC], BF16, kind="ExternalInput").ap()
    xB = nc.dram_tensor("xB", [128, KT, MC], BF16, kind="ExternalInput").ap()
    w3 = nc.dram_tensor("w3", [128, KT, 384], BF16, kind="ExternalInput").ap()
    wo = nc.dram_tensor("wo", [256, HID], BF16, kind="ExternalInput").ap()
    Cr = nc.dram_tensor("Cr", [128, S], BF16, kind="ExternalInput").ap()
    Sr = nc.dram_tensor("Sr", [128, S], BF16, kind="ExternalInput").ap()
    trimask = nc.dram_tensor("trimask", [128, 128], BF16,
                             kind="ExternalInput").ap()
    swapmr = nc.dram_tensor("swapm", [128, 128], BF16,
                            kind="ExternalInput").ap()
    yA = nc.dram_tensor("yA", [128, KT, MC], F16, kind="ExternalOutput").ap()
    yB = nc.dram_tensor("yB", [128, KT, MC], F16, kind="ExternalOutput").ap()
    dbg = {}
    if debug_dump:
        for nm, shp, dt in [("dqr0", [128, S], BF16), ("dqr1", [128, S], BF16),
                            ("dkr", [128, S], BF16), ("dva", [128, S], BF16),
                            ("dos0", [128, S], BF16), ("dos1", [128, S], BF16)]:
            dbg[nm] = nc.dram_tensor(nm, shp, dt, kind="ExternalOutput").ap()

    PHASE_BOUNDS.clear()
    mm_count = [0]

    def phase(name):
        PHASE_BOUNDS.append([name, mm_count[0]])
        mm_count[0] = 0

    def MM(*args, **kwargs):
        mm_count[0] += 1
        return nc.tensor.matmul(*args, **kwargs)

    def TP(*args, **kwargs):
        mm_count[0] += 1
        return nc.tensor.transpose(*args, **kwargs)

    with tile.TileContext(nc) as tc:
      for _pass in range(passes):
        with (
            tc.tile_pool(name="persist", bufs=1, side=None) as pers,
            tc.tile_pool(name="xpool", bufs=1) as xpool,
        ):
            # ---- persistent tiles ----
            qr = [pers.tile([128, S], BF16, tag=f"qr{t}", name=f"qr{t}")
                  for t in range(2)]
            kr = pers.tile([128, S], BF16, tag="kr")
            outstat = [pers.tile([128, S], BF16, tag=f"os{p}", name=f"os{p}")
                       for p in range(2)]
            wo_sb = [pers.tile([128, HID], BF16, tag=f"wo{p}", name=f"wo{p}")
                     for p in range(2)]
            Ct = pers.tile([128, S], BF16, tag="Ct")
            St = pers.tile([128, S], BF16, tag="St")
            wsb = pers.tile([128, KT * 384], BF16, tag="wsb")
            vaug = pers.tile([128, KT, 128], BF16, tag="vaug")
            trim = pers.tile([128, 128], BF16, tag="trim")
            swapm = pers.tile([128, 128], BF16, tag="swapm")
            ident = pers.tile([128, 128], BF16, tag="ident")
            wup = pers.tile([128, 128], BF16, tag="wup")

            xb0 = [xpool.tile([128, 4 * MC], BF16, tag=f"xb0_{b}",
                              name=f"xb0_{b}") for b in range(4)]
            xb1 = [xpool.tile([128, 4 * MC], BF16, tag=f"xb1_{b}",
                              name=f"xb1_{b}") for b in range(4)]

            # ---- preload DMAs: priority order IS service order (the DMA
            # engine pool serializes); slice the head of the stream so the
            # first matmuls can start ~4us in.  All bulk on HWDGE rings. ----
            W4 = 4 * 384
            nc.sync.dma_start(wsb[:, 0:W4], w3[:, 0:4, :])
            nc.scalar.dma_start(xb0[0][:, 0:2 * MC], xA[:, 0:2, :])
            nc.sync.dma_start(wsb[:, W4:2 * W4], w3[:, 4:8, :])
            nc.scalar.dma_start(xb0[0][:, 2 * MC:4 * MC], xA[:, 2:4, :])
            nc.sync.dma_start(xb0[1], xA[:, 4:8, :])
            nc.scalar.dma_start(wsb[:, 2 * W4:4 * W4], w3[:, 8:16, :])
            nc.sync.dma_start(xb0[2], xA[:, 8:12, :])
            nc.sync.dma_start(xb1[0], xB[:, 0:4, :])
            nc.scalar.dma_start(xb0[3], xA[:, 12:16, :])
            nc.scalar.dma_start(xb1[1], xB[:, 4:8, :])
            nc.sync.dma_start(Ct[:, 0:MC], Cr[:, 0:MC])
            nc.scalar.dma_start(St[:, 0:MC], Sr[:, 0:MC])
            nc.scalar.dma_start(swapm, swapmr)
            nc.sync.dma_start(xb1[2][:, 0:2 * MC], xB[:, 8:10, :])
            nc.scalar.dma_start(xb1[2][:, 2 * MC:4 * MC], xB[:, 10:12, :])
            nc.sync.dma_start(xb1[3][:, 0:2 * MC], xB[:, 12:14, :])
            # everything below rides the scalar ring so the sync queue is
            # empty when the rope-swap DMAs arrive (device round-robins
            # between queue heads)
            nc.scalar.dma_start(xb1[3][:, 2 * MC:4 * MC], xB[:, 14:16, :])
            nc.scalar.dma_start(trim, trimask)
            nc.scalar.dma_start(Ct[:, MC:S], Cr[:, MC:S])
            nc.scalar.dma_start(St[:, MC:S], Sr[:, MC:S])
            for p in range(2):
                nc.scalar.dma_start(wo_sb[p], wo[128 * p:128 * (p + 1), :])
            make_identity(nc, ident)
            nc.gpsimd.memset(vaug[:, :, 0:64], 1.0)

            # ====== pools ======
            qkv6 = tc.alloc_tile_pool(name="qkv6", bufs=1, space="PSUM")
            ps3 = [qkv6.tile([128, MC], F32, tag=f"ps3_{n}", name=f"ps3_{n}")
                   for n in range(3)]
            ps_n2 = [qkv6.tile([128, 512], F32, tag=f"m1n2_{c}",
                               name=f"m1n2_{c}") for c in range(2)]

            # ---- PE warmup: keep the clock hot while startup DMAs land
            # (vector memset is ready almost immediately) ----
            nc.vector.memset(wup, 0.0)
            for d in range(N_WARMUP):
                MM(ps3[0][:, 0:128], wup, wup, start=True, stop=True)
            phase("warmup")

            # ====== phase 1: QKV projection, m-chunk 0 ======
            NORD_LAST = (2, 0, 1)
            for b in range(8):
                for k in (2 * b, 2 * b + 1):
                    nord = NORD_LAST if k == KT - 1 else (0, 1, 2)
                    for n in nord:
                        for c in range(2):
                            MM(ps3[n][:, 512 * c:512 * (c + 1)],
                               wsb[:, 384 * k + 128 * n:
                                   384 * k + 128 * (n + 1)],
                               xb0[b // 2][:, (k % 4) * MC + 512 * c:
                                           (k % 4) * MC + 512 * (c + 1)],
                               start=(k == 0), stop=(k == KT - 1))
            phase("ph1_qkv")

            ev0 = tc.alloc_tile_pool(name="ev0", bufs=1)
            ev1 = tc.alloc_tile_pool(name="ev1", bufs=1)
            swp = tc.alloc_tile_pool(name="swp", bufs=3)
            qkvb0 = [ev0.tile([128, MC], BF16, tag=f"qkvb0_{t}",
                              name=f"qkvb0_{t}") for t in range(3)]
            qkvb1 = [ev1.tile([128, MC], BF16, tag=f"qkvb1_{t}",
                              name=f"qkvb1_{t}") for t in range(3)]

            def rope_muls(t, m0, qkvb, pe=False):
                """dst = qkv*C ; qbS = qkv*S ; swap via ring dma (default)
                or deferred to the PE (pe=True -> rope_add_pe)."""
                r0, r1 = (0, 128) if t < 2 else (64, 128)
                dst = qr[t] if t < 2 else kr
                qbS = swp.tile([128, MC], BF16, tag=f"qbS",
                               name=f"qbS{t}_{m0}")
                nc.vector.tensor_mul(dst[r0:r1, m0:m0 + MC],
                                     qkvb[t][r0:r1, 0:MC],
                                     Ct[r0:r1, m0:m0 + MC])
                # full 128 rows even for k (rows 0:64 = v * sin, discarded):
                # keeps the PE-swap moving operand fully initialized
                nc.gpsimd.tensor_mul(qbS[:, 0:MC], qkvb[t][:, 0:MC],
                                     St[:, m0:m0 + MC])
                if pe:
                    return qbS
                sw = swp.tile([128, MC], BF16, tag=f"sw",
                              name=f"sw{t}_{m0}")
                for g in range(r0 // 32, r1 // 32, 2):
                    nc.sync.dma_start(sw[32 * g:32 * g + 32, :],
                                      qbS[32 * g + 32:32 * g + 64, :])
                    nc.sync.dma_start(sw[32 * g + 32:32 * g + 64, :],
                                      qbS[32 * g:32 * g + 32, :])
                return sw

            def rope_add(t, m0, sw, kdup_eng=None):
                r0, r1 = (0, 128) if t < 2 else (64, 128)
                dst = qr[t] if t < 2 else kr
                nc.vector.tensor_add(dst[r0:r1, m0:m0 + MC],
                                     dst[r0:r1, m0:m0 + MC], sw[r0:r1, :])
                if t == 2:
                    (kdup_eng or nc.sync).dma_start(kr[0:64, m0:m0 + MC],
                                                    kr[64:128, m0:m0 + MC])

            def rope_add_pe(t, m0, qbS, targets, kdup_eng=None):
                """swap on the PE (permutation matmul into freed ph1 psum) +
                DVE add from PSUM — keeps the startup rope off the congested
                DMA device."""
                r0, r1 = (0, 128) if t < 2 else (64, 128)
                dst = qr[t] if t < 2 else kr
                for c in range(2):
                    sw = targets[c]
                    MM(sw, swapm, qbS[:, 512 * c:512 * (c + 1)],
                       start=True, stop=True)
                    nc.vector.tensor_add(
                        dst[r0:r1, m0 + 512 * c:m0 + 512 * (c + 1)],
                        dst[r0:r1, m0 + 512 * c:m0 + 512 * (c + 1)],
                        sw[r0:r1, 0:512])
                if t == 2:
                    (kdup_eng or nc.sync).dma_start(kr[0:64, m0:m0 + MC],
                                                    kr[64:128, m0:m0 + MC])

            def qkv_m1_unit(n, c, kq, ps, evict=True):
                """4 accumulating matmuls (quarter k-tiles) for m-chunk 1."""
                for k in range(4 * kq, 4 * kq + 4):
                    MM(ps[:, 0:512],
                       wsb[:, 384 * k + 128 * n:384 * k + 128 * (n + 1)],
                       xb1[k // 4][:, (k % 4) * MC + 512 * c:
                                   (k % 4) * MC + 512 * (c + 1)],
                       start=(k == 0), stop=(k == KT - 1))
                if evict and kq == 3:
                    nc.vector.tensor_copy(qkvb1[n][:, 512 * c:512 * (c + 1)],
                                          ps[:, 0:512])

            # ---- m-chunk-0 evict + rope (vector/scalar/pool), interleaved
            # with the m-chunk-1 k/v group on the PE.  The rope swaps are
            # permutation matmuls into ph1 psum banks freed by the evicts —
            # nothing rides the (saturated) DMA device. ----
            nc.vector.tensor_copy(qkvb0[2], ps3[2])       # k|v first
            sw_k = rope_muls(2, 0, qkvb0, pe=True)
            nc.scalar.copy(qkvb0[0], ps3[0])
            sw_q0 = rope_muls(0, 0, qkvb0, pe=True)
            nc.scalar.copy(qkvb0[1], ps3[1])
            sw_q1 = rope_muls(1, 0, qkvb0, pe=True)
            for c in range(2):
                qkv_m1_unit(2, c, 0, ps_n2[c], evict=False)
            rope_add_pe(2, 0, sw_k,
                        [ps3[2][:, 0:512], ps3[2][:, 512:1024]])
            for c in range(2):
                qkv_m1_unit(2, c, 1, ps_n2[c], evict=False)
            rope_add_pe(0, 0, sw_q0,
                        [ps3[0][:, 0:512], ps3[0][:, 512:1024]])
            for c in range(2):
                qkv_m1_unit(2, c, 2, ps_n2[c], evict=False)
            rope_add_pe(1, 0, sw_q1,
                        [ps3[1][:, 0:512], ps3[1][:, 512:1024]])
            for c in range(2):
                qkv_m1_unit(2, c, 3, ps_n2[c], evict=False)
            for c in range(2):
                nc.vector.tensor_copy(qkvb1[2][:, 512 * c:512 * (c + 1)],
                                      ps_n2[c][:, 0:512])
            phase("m1_n2")

            qkv6.release()

            # ====== attention pools (after qkv6 release: 8 PSUM banks).
            # fill sits at the bottom of the stack so stp+pvp can be
            # released before the tail for a deeper psum rotation. ======
            fill = tc.alloc_tile_pool(name="fill", bufs=2, space="PSUM")
            stp = tc.alloc_tile_pool(name="stp", bufs=2, space="PSUM")
            pvp = tc.alloc_tile_pool(name="pvp", bufs=1, space="PSUM")
            ptp = tc.alloc_tile_pool(name="ptp", bufs=4)
            nrm = tc.alloc_tile_pool(name="nrm", bufs=2)
            ysbp = tc.alloc_tile_pool(name="ysbp", bufs=8)

            def transposes(mc, qkvb2):
                """v^T into vaug[:, i, 64:128]: 8 transposes into one PSUM
                tile back-to-back, then a single strided DVE copy."""
                tpb = fill.tile([128, 8, 64], BF16, tag="fl", name=f"tpb{mc}")
                for i in range(8):
                    TP(tpb[:, i, :], qkvb2[0:64, 128 * i:128 * (i + 1)],
                       ident[0:64, 0:64])
                nc.vector.tensor_copy(vaug[:, 8 * mc:8 * (mc + 1), 64:128],
                                      tpb)

            # ---- filler queue consumed inside J0's attention; ordered so
            # everything J1's first heads need comes out early ----
            fillq = [
                lambda: transposes(0, qkvb0[2]),
                lambda: rope_add(2, MC, rope_muls(2, MC, qkvb1),
                                 kdup_eng=nc.scalar),
                lambda: transposes(1, qkvb1[2]),
            ]

            def m1_units(n):
                for c in range(2):
                    ps_h = {}

                    def unit(kq, n=n, c=c, ps_h=ps_h):
                        if kq == 0:
                            ps_h[0] = fill.tile([128, 512], F32, tag="fl",
                                                name=f"m1_{n}_{c}")
                        qkv_m1_unit(n, c, kq, ps_h[0])

                    for kq in range(4):
                        fillq.append(lambda u=unit, kq=kq: u(kq))

            m1_units(0)
            fillq.append(lambda: rope_add(0, MC, rope_muls(0, MC, qkvb1)))
            m1_units(1)
            fillq.append(lambda: rope_add(1, MC, rope_muls(1, MC, qkvb1)))

            def filler(_slot):
                if fillq:
                    fillq.pop(0)()

            def norm_chunk(j, h, pv, cc, tiles, move_eng=None,
                           fast_tail=False):
                """Normalize cols [512cc:512cc+512] of pv.  Emitted inline
                as soon as the last PV block writing those cols is issued
                (cols 0:512c+512 are final after i = 8j + 4cc + 3)."""
                half, p = h % 2, h // 2
                me = move_eng or nc.gpsimd
                q0 = QCHUNK * j
                rec, pvs, tmp = tiles
                sl = slice(512 * cc, 512 * (cc + 1))
                osl = slice(q0 + 512 * cc, q0 + 512 * (cc + 1))
                nc.vector.reciprocal_approx_fast(rec[0:64, sl], pv[0:64, sl])
                me.dma_start(rec[64:128, sl], rec[0:64, sl])
                if fast_tail:
                    assert half
                    nc.vector.tensor_mul(outstat[p][64:128, osl],
                                         pv[64:128, sl], rec[64:128, sl])
                    return
                nc.vector.tensor_copy(pvs[64:128, sl], pv[64:128, sl])
                if half:
                    nc.gpsimd.tensor_mul(outstat[p][64:128, osl],
                                         pvs[64:128, sl], rec[64:128, sl])
                else:
                    nc.gpsimd.tensor_mul(tmp[64:128, sl],
                                         pvs[64:128, sl], rec[64:128, sl])
                    me.dma_start(outstat[p][0:64, osl], tmp[64:128, sl])

            def emit_attn(j, h, absorber=None, norm_move=None,
                          fast_tail=False):
                half, p = h % 2, h // 2
                qrow = 64 * half
                kb = 64 * half
                ilast = 8 * (j + 1) - 1
                pv = pvp.tile([128, QCHUNK], F32, tag="pv", name=f"pv{j}_{h}")
                rec = nrm.tile([128, QCHUNK], F32, tag="rec",
                               name=f"rec{j}_{h}")
                tmp = None
                pvs = nrm.tile([128, QCHUNK], BF16, tag="pvs",
                               name=f"pvs{j}_{h}")
                if not half:
                    tmp = nrm.tile([128, QCHUNK], BF16, tag="tmp",
                                   name=f"tmp{j}_{h}")
                ntile = (rec, pvs, tmp)
                # triangle blocks packed in complementary pairs so one exp
                # call covers a full [128,1024] tile (fewer scalar-engine
                # fixed overheads; scalar paces J1)
                if j == 0:
                    groups = [[0], [1, 7], [2, 6], [3, 5], [4]]
                    slots = {1, 2, 3, 4}
                    norm0_g = 3          # group after which cols 0:512 final
                else:
                    groups = [[i] for i in range(9)] + \
                        [[9, 15], [10, 14], [11, 13], [12]]
                    slots = {1, 3, 5, 7, 9, 10, 11, 12}
                    norm0_g = 11
                ab = 0

                def seg_chunks(a, b):
                    """split [a,b) at multiples of 512 (psum bank bounds)"""
                    out = []
                    while a < b:
                        e = min(b, (a // 512 + 1) * 512)
                        out.append((a, e - a))
                        a = e
                    return out

                def emit_pvs(gi, metas, pt):
                    for (i, o0, qstart, qlen) in metas:
                        diag = 128 * i >= QCHUNK * j
                        off = qstart - QCHUNK * j
                        chunks = seg_chunks(off, off + qlen)
                        # emit the non-masked chunk first (hides mask latency)
                        if diag and len(chunks) > 1:
                            chunks = chunks[::-1]
                        for (a, cols) in chunks:
                            MM(pv[:, a:a + cols],
                               vaug[:, i, :],
                               pt[:, o0 + (a - off):o0 + (a - off) + cols],
                               start=(gi == 0), stop=(gi == len(groups) - 1))
                    if gi == norm0_g:
                        norm_chunk(j, h, pv, 0, ntile, move_eng=norm_move)
                    elif gi == len(groups) - 1:
                        norm_chunk(j, h, pv, 1, ntile, move_eng=norm_move,
                                   fast_tail=fast_tail)

                pending = None   # PV emission runs one group behind QK/exp
                for gi, grp in enumerate(groups):
                    metas = []
                    ot = 0
                    st = stp.tile([128, 1024], F32, tag="st",
                                  name=f"st{j}_{h}_{gi}")
                    pt = ptp.tile([128, 1024], BF16, tag="pt",
                                  name=f"pt{j}_{h}_{gi}")
                    for i in grp:
                        qstart = max(QCHUNK * j, 128 * i)
                        qlen = QCHUNK * (j + 1) - qstart
                        for (a, cols) in seg_chunks(ot, ot + qlen):
                            MM(st[:, a:a + cols],
                               kr[kb:kb + 64, 128 * i:128 * (i + 1)],
                               qr[p][qrow:qrow + 64,
                                     qstart + (a - ot):
                                     qstart + (a - ot) + cols],
                               start=True, stop=True)
                        metas.append((i, ot, qstart, qlen))
                        ot += qlen
                    nc.scalar.activation(
                        pt[:, 0:ot], st[:, 0:ot],
                        mybir.ActivationFunctionType.Exp, scale=SCALE)
                    for (i, o0, qstart, qlen) in metas:
                        if 128 * i >= QCHUNK * j:
                            nc.vector.tensor_mul(pt[:, o0:o0 + 128],
                                                 pt[:, o0:o0 + 128], trim)
                    if absorber is not None and gi in slots:
                        absorber(ab)
                        ab += 1
                    if pending is not None:
                        emit_pvs(*pending)
                    pending = (gi, metas, pt)
                emit_pvs(*pending)

            # ====== J0: attention q-chunk 0, fillers interleaved ======
            for hi, h in enumerate(J0_ORDER):
                emit_attn(0, h, absorber=filler,
                          norm_move=(nc.sync if hi == 3 else None))
                phase(f"J0_h{h}")
            while fillq:
                fillq.pop(0)()
            phase("J0_fill_rest")

            # ====== J1 with o_proj mcol=0 interleaved ======
            ysb_jobs = {}

            def oproj_job(nt, c, mcol, cp_eng, store_eng, pool=None,
                          half_store=False):
                """One [128,512] o_proj chunk: 2 matmuls + fp16 copy + store."""
                pso = (pool or fill).tile([128, 512], F32,
                                          tag=("fl" if (pool or fill) is fill
                                               else "tl"),
                                          name=f"pso{nt}_{c}_{mcol}")
                for p in range(2):
                    MM(pso[:, 0:512],
                       wo_sb[p][:, 128 * nt:128 * (nt + 1)],
                       outstat[p][:, mcol + 512 * c:mcol + 512 * (c + 1)],
                       start=(p == 0), stop=(p == 1))
                pair, slot = nt // 2, nt % 2
                ysbt = ysb_jobs.get((pair, mcol))
                if ysbt is None:
                    ysbt = ysbp.tile([128, 2, MC], F16, tag="ysb",
                                     name=f"ysb{pair}_{mcol}")
                    ysb_jobs[(pair, mcol)] = ysbt
                if cp_eng is nc.scalar:
                    nc.scalar.copy(ysbt[:, slot, 512 * c:512 * (c + 1)],
                                   pso[:, 0:512])
                else:
                    cp_eng.tensor_copy(ysbt[:, slot, 512 * c:512 * (c + 1)],
                                       pso[:, 0:512])
                ydst = yA if mcol == 0 else yB
                if half_store:
                    # c-major sweeps: store this 512-col half of the pair as
                    # soon as both slots have it.
                    if slot == 1:
                        store_eng.dma_start(
                            ydst[:, 2 * pair:2 * pair + 2,
                                 512 * c:512 * (c + 1)],
                            ysbt[:, :, 512 * c:512 * (c + 1)])
                        if c == 1:
                            del ysb_jobs[(pair, mcol)]
                elif slot == 1 and c == 1:
                    store_eng.dma_start(ydst[:, 2 * pair:2 * pair + 2, :],
                                        ysbt)
                    del ysb_jobs[(pair, mcol)]

            for hi, h in enumerate(J1_ORDER):
                jobs = [(nt, c)
                        for nt in range(4 * hi, 4 * hi + 4) for c in range(2)]

                def absorber(ab, jobs=jobs, hi=hi):
                    # hi==0: outstat j0 cols land late (J0's last heads);
                    # don't let an early oproj mm block the PE FIFO.
                    # hi==3: evict on scalar so DVE is free for the last
                    # head's low-latency normalize; keep the last 4 jobs to
                    # run during the normalize latency (PE stays warm).
                    ev = nc.vector
                    if hi == 0:
                        if ab < 4:
                            return
                        for jb in (2 * (ab - 4), 2 * (ab - 4) + 1):
                            nt, c = jobs[jb]
                            oproj_job(nt, c, 0, ev, nc.sync)
                    elif hi == 3 and ab >= 4:
                        return
                    else:
                        nt, c = jobs[ab]
                        oproj_job(nt, c, 0, ev, nc.sync)

                emit_attn(1, h, absorber=absorber,
                          norm_move=(nc.sync if hi == 3 else None),
                          fast_tail=(hi == 3))
                if hi == 3:
                    for jb in range(4, 8):
                        nt, c = jobs[jb]
                        oproj_job(nt, c, 0, nc.scalar, nc.sync)
                phase(f"J1_h{h}")

            # ====== o_proj mcol=1024 tail: release pvp (its boundary waits
            # only the last pv readers) -> 2 extra banks; 4-deep rotation
            # across fill+tailp.  c-major sweeps (c=0 jobs only need the
            # last head's first normalize chunk); half-stores per sweep so
            # the output DMA is spread, not bunched at the end. ======
            pvp.release()
            tailp = tc.alloc_tile_pool(name="tailp", bufs=2, space="PSUM")

            for c in range(2):
                for nt in range(KT):
                    pool = fill if (nt < 2 or nt % 2 == 1) else tailp
                    oproj_job(nt, c, MC,
                              nc.scalar if nt % 2 == 0 else nc.vector,
                              nc.sync, pool=pool, half_store=True)
            phase("tail_oproj")

            if debug_dump:
                nc.sync.dma_start(dbg["dqr0"], qr[0])
                nc.sync.dma_start(dbg["dqr1"], qr[1])
                nc.sync.dma_start(dbg["dkr"], kr)
                nc.sync.dma_start(dbg["dva"], vaug)
                nc.sync.dma_start(dbg["dos0"], outstat[0])
                nc.sync.dma_start(dbg["dos1"], outstat[1])

            ysbp.release()
            nrm.release()
            ptp.release()
            tailp.release()
            stp.release()
            fill.release()
            swp.release()
            ev1.release()
            ev0.release()

    nc.compile()
    return nc


def make_host_inputs(x, w_qkv, w_o):
    """Host-side prep: tiled/transposed bf16 inputs, rope tables."""
    import ml_dtypes
    bf16 = ml_dtypes.bfloat16
    x = np.asarray(x, dtype=np.float32)
    w_qkv = np.asarray(w_qkv, dtype=np.float32)
    w_o = np.asarray(w_o, dtype=np.float32)
    xT = np.ascontiguousarray(x.reshape(S, HID).T)          # [HID, S]
    xT3 = xT.reshape(KT, 128, S).transpose(1, 0, 2)
    xAh = np.ascontiguousarray(xT3[:, :, 0:MC]).astype(bf16)
    xBh = np.ascontiguousarray(xT3[:, :, MC:S]).astype(bf16)

    inv_freq = 1.0 / (ROPE_BASE ** (np.arange(0, HD, 2, dtype=np.float32) / HD))
    t = np.arange(S, dtype=np.float32)
    freqs = np.outer(t, inv_freq)                     # [S, 32]
    cosT = np.cos(freqs).T.astype(np.float32)         # [32, S]
    sinT = np.sin(freqs).T.astype(np.float32)
    C = np.ascontiguousarray(np.tile(cosT, (4, 1))).astype(bf16)   # [128, S]
    # S is applied BEFORE the row swap, so the sign pattern rides along with
    # the swap: rows 0:32 = +sin, 32:64 = -sin (swapped vs the classic table).
    Sn = np.ascontiguousarray(np.tile(np.concatenate([sinT, -sinT], 0),
                                      (2, 1))).astype(bf16)

    r = np.arange(128)
    trimask = np.where(r[None, :] >= r[:, None], np.float32(1.0),
                       np.float32(0.0)).astype(bf16)
    swapm = np.where(r[None, :] == (r[:, None] ^ 32), np.float32(1.0),
                     np.float32(0.0)).astype(bf16)

    in_maps = []
    for c in range(NCORES):
        qcols = np.arange(4 * c * HD, 4 * (c + 1) * HD)
        vcols = NH * HD + NKV * HD + np.arange(c * HD, (c + 1) * HD)
        kcols = NH * HD + np.arange(c * HD, (c + 1) * HD)
        w_stat = np.concatenate(
            [w_qkv[:, qcols], w_qkv[:, vcols], w_qkv[:, kcols]], axis=1)
        w3c = np.ascontiguousarray(
            w_stat.reshape(KT, 128, 384).transpose(1, 0, 2)).astype(bf16)
        wo_c = np.ascontiguousarray(
            w_o[256 * c:256 * (c + 1), :]).astype(bf16)
        in_maps.append({
            "xA": xAh, "xB": xBh, "w3": w3c, "wo": wo_c,
            "Cr": C, "Sr": Sn, "trimask": trimask, "swapm": swapm,
        })
    return in_maps


_NC_CACHE = {}


def get_nc():
    if "nc" not in _NC_CACHE:
        _NC_CACHE["nc"] = build_kernel()
    return _NC_CACHE["nc"]


def _get_exec():
    """Build (once) the jitted sharded executable over the 8 cores."""
    if "exec" in _NC_CACHE:
        return _NC_CACHE["exec"]
    import jax
    from jax.sharding import Mesh, PartitionSpec, NamedSharding
    from jax.experimental.shard_map import shard_map
    from concourse import bass2jax

    nc = get_nc()
    bass2jax.install_neuronx_cc_hook()
    partition_name = (nc.partition_id_tensor.name
                      if nc.partition_id_tensor else None)
    in_names, out_names, out_avals, zero_outs = [], [], [], []
    for alloc in nc.m.functions[0].allocations:
        if not isinstance(alloc, mybir.MemoryLocationSet):
            continue
        name = alloc.memorylocations[0].name
        if alloc.kind == "ExternalInput":
            if name != partition_name:
                in_names.append(name)
        elif alloc.kind == "ExternalOutput":
            shape = tuple(alloc.tensor_shape)
            dtype = mybir.dt.np(alloc.dtype)
            out_names.append(name)
            out_avals.append(jax.core.ShapedArray(shape, dtype))
            zero_outs.append(np.zeros(shape, dtype))
    n_params = len(in_names)
    all_in = list(in_names) + list(out_names)
    if partition_name is not None:
        all_in.append(partition_name)

    def _body(*args):
        operands = list(args)
        if partition_name is not None:
            operands.append(bass2jax.partition_id_tensor())
        return tuple(bass2jax._bass_exec_p.bind(
            *operands, out_avals=tuple(out_avals), in_names=tuple(all_in),
            out_names=tuple(out_names), lowering_input_output_aliases=(),
            sim_require_finite=True, sim_require_nnan=True, nc=nc))

    devices = jax.devices()[:NCORES]
    mesh = Mesh(np.asarray(devices), ("core",))
    REPL = {"xA", "xB", "Cr", "Sr", "trimask", "swapm"}
    in_specs = tuple(PartitionSpec() if n in REPL else PartitionSpec("core")
                     for n in in_names)
    in_specs = in_specs + (PartitionSpec("core"),) * len(out_names)
    f = jax.jit(shard_map(_body, mesh=mesh, in_specs=in_specs,
                          out_specs=(PartitionSpec("core"),) * len(out_names),
                          check_rep=False), keep_unused=True)
    sh = NamedSharding(mesh, PartitionSpec("core"))
    shr = NamedSharding(mesh, PartitionSpec())
    _NC_CACHE["exec"] = (f, in_names, out_names, zero_outs, sh, shr, REPL)
    return _NC_CACHE["exec"]


def kernel(x, w_qkv, w_o):
    import jax

    f, in_names, out_names, zero_outs, sh, shr, REPL = _get_exec()
    in_maps = make_host_inputs(x, w_qkv, w_o)
    args = []
    for name in in_names:
        if name in REPL:
            args.append(jax.device_put(in_maps[0][name], shr))
        else:
            args.append(jax.device_put(
                np.concatenate([m[name] for m in in_maps], 0), sh))
    if "zeros" not in _NC_CACHE:
        _NC_CACHE["zeros"] = [
            jax.device_put(
                np.zeros((NCORES * z.shape[0], *z.shape[1:]), z.dtype), sh)
            for z in zero_outs]
    args += _NC_CACHE["zeros"]
    outs = f(*args)
    ya_idx = out_names.index("yA")
    yb_idx = out_names.index("yB")
    if "reduce" not in _NC_CACHE:
        import jax.numpy as jnp

        def _reduce(a, b):
            # a,b: [8*128, KT, MC] fp16 partials -> [S, HID] f32
            y = jnp.concatenate(
                [jnp.reshape(a, (NCORES, 128, KT, MC)),
                 jnp.reshape(b, (NCORES, 128, KT, MC))], axis=3)
            y = jnp.sum(y.astype(jnp.float32), axis=0)     # [128, KT, S]
            y = jnp.transpose(y, (1, 0, 2))                # [KT, 128, S]
            return jnp.transpose(jnp.reshape(y, (HID, S)))  # [S, HID]
        _NC_CACHE["reduce"] = jax.jit(_reduce)
    out = np.asarray(_NC_CACHE["reduce"](outs[ya_idx], outs[yb_idx]))
    return np.ascontiguousarray(out.astype(np.float32)).reshape(B, S, HID)
